# revision 1
# baseline (speedup 1.0000x reference)
"""CrossNetV2 soft-MoE kernel for 8 axon-tunneled TRN2 NeuronCores.

Problem (hardcoded shapes): B=16384, D=1024, R=64, E=4, L=3.
    for l in range(L):
        h         = relu(einsum('bd,edr->ber', x, U[l]))          # [B,E,R]
        expert    = einsum('ber,erd->bed', h, V[l])               # [B,E,D]
        gate      = softmax(x @ gW[l] + gb[l])                    # [B,E]
        mixed     = einsum('be,bed->bd', gate, expert)            # [B,D]
        x         = x0 * mixed + bias[l] + x

Strategy:
  - Data parallel: batch split 8 ways (2048 rows/core); params replicated.
  - Feature-on-partitions layout throughout; host transposes x0 to [D, B_core].
  - Running-sum reformulation: x_l = x0 * M_l + B_l with
        M_l = 1 + sum_{j<=l} mixed_j,   B_l = sum_{j<=l} bias_j  (host const)
    so the per-layer epilogue is only:  M += mixed (DVE);  xb = bf16(M * x0)
    (GpSimd, chunk-paired). The affine shift B_{l-1} is folded into the next
    layer's matmuls via per-partition bias APs on the relu/exp activations
    (B@U on relu, B@gW + gb on exp) at zero cost. On the last layer M is
    preloaded into the PSUM bank and the V-matmuls accumulate on top
    (start=False), so the final add is free; y = M_f*x0 + B_2 via DVE + ACT.
  - Gate folded into h before the V contraction (mixed = (gate_bc*relu(h)) @ V);
    softmax denominator via ones-matmul, reciprocal_approx_fast on DVE,
    broadcasts back to 4/128 partitions via tiny PE matmuls.
  - Matmuls in bf16 (PSUM accumulates f32); M kept in f32 (precision anchor;
    bf16 rounding enters per layer but never compounds across layers).
  - Software pipelining: layer-major over the 4 batch tiles with ~1.5-phase
    lookahead; every PE op of the softmax chain for item i is emitted behind
    GEMM work of item i+1, so its cross-engine dependency has resolved when
    the PE reaches it. UNROLL=8 invocations are chained into one continuous
    pipeline per For_i iteration (the all-engine loop barrier costs ~16-25us,
    paid once per 8 invocations).
"""

import os
import sys

sys.path.insert(0, "/opt/trn_rl_repo")

import numpy as np

B, D, R, E, L = 16384, 1024, 64, 4, 3
N_CORES = 8
BC = B // N_CORES          # rows per core
BT = 512                   # batch-tile (free dim / PSUM bank width)
NBT = BC // BT             # b-tiles per core
ER = E * R                 # 256
DC = D // 128              # 8 d-chunks
EC = ER // 128             # 2 er-chunks

MM_DTYPE = os.environ.get("KMM_DTYPE", "bf16")   # "bf16" | "f32r"
XB_GPS = int(os.environ.get("KXB_GPS", "8"))     # d-chunks of xb-mul on GpSimd


def build_body(nc, tc, reps=1):
    import concourse.mybir as mybir

    f32 = mybir.dt.float32
    f32r = mybir.dt.float32r
    wdt = mybir.dt.bfloat16 if MM_DTYPE == "bf16" else f32r
    AF = mybir.ActivationFunctionType

    x0T = nc.dram_tensor("x0T", [D, BC], wdt, kind="ExternalInput")
    U_all = nc.dram_tensor("U_all", [L, D, ER], wdt, kind="ExternalInput")
    V_all = nc.dram_tensor("V_all", [L, ER, D], wdt, kind="ExternalInput")
    gW_all = nc.dram_tensor("gW_all", [L, D, E], wdt, kind="ExternalInput")
    sel_h = nc.dram_tensor("sel_h", [E, EC, 128], wdt, kind="ExternalInput")
    ones_h = nc.dram_tensor("ones_h", [4, 4], wdt, kind="ExternalInput")
    gbB_h = nc.dram_tensor("gbB_h", [E, L], f32, kind="ExternalInput")
    BU_h = nc.dram_tensor("BU_h", [128, L, EC], f32, kind="ExternalInput")
    B2_h = nc.dram_tensor("B2_h", [128, DC], f32, kind="ExternalInput")
    yT = nc.dram_tensor("yT", [D, BC], f32, kind="ExternalOutput")

    from contextlib import ExitStack

    ctx = ExitStack()
    const = ctx.enter_context(tc.tile_pool(name="const", bufs=1))
    x0p = ctx.enter_context(tc.tile_pool(name="x0p", bufs=NBT))
    Mp = ctx.enter_context(tc.tile_pool(name="Mp", bufs=NBT))
    xbp = ctx.enter_context(tc.tile_pool(name="xbp", bufs=NBT))
    hrp = ctx.enter_context(tc.tile_pool(name="hrp", bufs=4))
    hsp = ctx.enter_context(tc.tile_pool(name="hsp", bufs=3))
    smp = ctx.enter_context(tc.tile_pool(name="smp", bufs=3))
    gnp = ctx.enter_context(tc.tile_pool(name="gnp", bufs=2))
    tp = ctx.enter_context(tc.tile_pool(name="tp", bufs=3))
    yp = ctx.enter_context(tc.tile_pool(name="yp", bufs=2))
    ps_lg = ctx.enter_context(tc.tile_pool(name="ps_lg", bufs=3, space="PSUM"))
    ps_h = ctx.enter_context(tc.tile_pool(name="ps_h", bufs=2, space="PSUM"))
    ps_mx = ctx.enter_context(tc.tile_pool(name="ps_mx", bufs=3, space="PSUM"))

    # ---- constants (loaded once, outside the reps loop) ----
    Ur = const.tile([128, L, DC, ER], wdt)
    Vr = const.tile([128, L, EC, D], wdt)
    gWr = const.tile([128, L, DC, E], wdt)
    selr = const.tile([E, EC, 128], wdt)
    ones44 = const.tile([4, 4], wdt)
    gbBt = const.tile([E, L], f32)
    BUr = const.tile([128, L, EC], f32)
    B2t = const.tile([128, DC], f32)

    # Small/layer-0 params first so a cold (single-invocation) run can start
    # the first logits/h matmuls before the later layers' weights land.
    nc.sync.dma_start(out=gWr, in_=gW_all.rearrange("l (c p) e -> p l c e", p=128))
    nc.sync.dma_start(out=selr, in_=sel_h[:, :, :])
    nc.sync.dma_start(out=ones44, in_=ones_h[:, :])
    nc.sync.dma_start(out=gbBt, in_=gbB_h[:, :])
    nc.sync.dma_start(out=BUr, in_=BU_h[:, :, :])
    nc.sync.dma_start(out=B2t, in_=B2_h[:, :])
    for l in range(L):
        nc.sync.dma_start(
            out=Ur[:, l],
            in_=U_all[l].rearrange("(c p) er -> p c er", p=128),
        )
        nc.sync.dma_start(
            out=Vr[:, l],
            in_=V_all[l].rearrange("(k p) d -> p k d", p=128),
        )

    # persistent per-tile state
    x0_t = [x0p.tile([128, DC, BT], wdt, tag="x0", name=f"x0_{i}") for i in range(NBT)]
    M_t = [Mp.tile([128, DC, BT], f32, tag="M", name=f"M_{i}") for i in range(NBT)]
    xb_t = [xbp.tile([128, DC, BT], wdt, tag="xb", name=f"xb_{i}") for i in range(NBT)]

    state = {}

    def ph_logits(key):
        r, l, t = key
        b0 = t * BT
        if l == 0:
            nc.sync.dma_start(
                out=x0_t[t],
                in_=x0T[:, b0 : b0 + BT].rearrange("(c p) b -> p c b", p=128),
            )
        xin = x0_t[t] if l == 0 else xb_t[t]
        logits = ps_lg.tile([E, BT], f32, tag="lg")
        for c in range(DC):
            nc.tensor.matmul(
                logits,
                gWr[:, l, c, :],
                xin[:, c, :],
                start=(c == 0),
                stop=(c == DC - 1),
            )
        explog = smp.tile([E, BT], wdt, tag="explog")
        nc.scalar.activation(
            out=explog, in_=logits, func=AF.Exp, bias=gbBt[:, l : l + 1], scale=1.0
        )
        state[key] = {"explog": explog, "hrs": [None, None]}

    def ph_h(key, m):
        r, l, t = key
        xin = x0_t[t] if l == 0 else xb_t[t]
        hm = ps_h.tile([128, BT], f32, tag="hp")
        for c in range(DC):
            nc.tensor.matmul(
                hm,
                Ur[:, l, c, m * 128 : (m + 1) * 128],
                xin[:, c, :],
                start=(c == 0),
                stop=(c == DC - 1),
            )
        hr = hrp.tile([128, BT], wdt, tag="hr")
        nc.scalar.activation(
            out=hr, in_=hm, func=AF.Relu, bias=BUr[:, l, m : m + 1], scale=1.0
        )
        state[key]["hrs"][m] = hr

    def ph_sum(key):
        r, l, t = key
        st = state[key]
        s = ps_lg.tile([1, BT], f32, tag="lg")
        nc.tensor.matmul(s, ones44[:, 0:1], st["explog"], start=True, stop=True)
        recip = smp.tile([1, BT], f32, tag="recip")
        nc.vector.reciprocal_approx_fast(out=recip, in_=s)
        recip_b = smp.tile([1, BT], wdt, tag="recipb")
        nc.scalar.copy(out=recip_b, in_=recip)
        st["recip_b"] = recip_b

    def ph_r4(key):
        st = state[key]
        r4 = ps_lg.tile([E, BT], f32, tag="lg")
        nc.tensor.matmul(r4, ones44[0:1, :], st["recip_b"], start=True, stop=True)
        gate_n = gnp.tile([E, BT], wdt, tag="gate_n")
        with nc.allow_low_precision(reason="gate in matmul dtype"):
            nc.vector.tensor_mul(out=gate_n, in0=st["explog"], in1=r4)
        st["gate_n"] = gate_n

    def ph_gbc(key):
        r, l, t = key
        st = state[key]
        hs = hsp.tile([128, EC, BT], wdt, tag="hs")
        for m in range(EC):
            gbc = ps_mx.tile([128, BT], f32, tag="mx")
            nc.tensor.matmul(gbc, selr[:, m, :], st["gate_n"], start=True, stop=True)
            with nc.allow_low_precision(reason="hs in matmul dtype"):
                nc.vector.tensor_mul(out=hs[:, m, :], in0=st["hrs"][m], in1=gbc)
        st["hs"] = hs
        if l == L - 1:
            # pre-allocate + preload the first mixed bank one phase early so
            # the PE's first V-matmul doesn't wait on the ACT copy.
            mx0 = ps_mx.tile([128, BT], f32, tag="mx")
            nc.scalar.copy(out=mx0, in_=M_t[t][:, 0, :])
            st["mx0"] = mx0

    def ph_mix(key):
        r, l, t = key
        b0 = t * BT
        st = state.pop(key)
        hs = st["hs"]
        last = l == L - 1
        for d in range(DC):
            if last and d == 0:
                mx = st["mx0"]
            else:
                mx = ps_mx.tile([128, BT], f32, tag="mx")
            if last and d > 0:
                # preload M into the PSUM bank; the V-matmuls accumulate on
                # top, yielding M_final = M + mixed with no extra DVE add.
                nc.scalar.copy(out=mx, in_=M_t[t][:, d, :])
            for k in range(EC):
                nc.tensor.matmul(
                    mx,
                    Vr[:, l, k, d * 128 : (d + 1) * 128],
                    hs[:, k, :],
                    start=(k == 0 and not last),
                    stop=(k == EC - 1),
                    skip_group_check=last,
                )
            if last:
                td = tp.tile([128, BT], f32, tag="t")
                nc.vector.tensor_mul(out=td, in0=mx, in1=x0_t[t][:, d, :])
                yd = yp.tile([128, BT], f32, tag="y")
                nc.scalar.activation(
                    out=yd, in_=td, func=AF.Identity, bias=B2t[:, d : d + 1], scale=1.0
                )
                nc.sync.dma_start(
                    out=yT[:, b0 : b0 + BT].rearrange("(c p) b -> p c b", p=128)[
                        :, d, :
                    ],
                    in_=yd,
                )
            else:
                if l == 0:
                    # M = mixed + 1  (f32 anchor)
                    nc.scalar.activation(
                        out=M_t[t][:, d, :], in_=mx, func=AF.Identity,
                        bias=1.0, scale=1.0,
                    )
                else:
                    nc.vector.tensor_add(
                        out=M_t[t][:, d, :], in0=M_t[t][:, d, :], in1=mx
                    )
                if d % 2 == 1:
                    # one fused [128, 2, BT] mul per d-pair (amortizes the
                    # per-op fixed cost on the engine)
                    eng = nc.gpsimd if d < XB_GPS else nc.vector
                    with nc.allow_low_precision(reason="xb in matmul dtype"):
                        eng.tensor_mul(
                            out=xb_t[t][:, d - 1 : d + 1, :],
                            in0=M_t[t][:, d - 1 : d + 1, :],
                            in1=x0_t[t][:, d - 1 : d + 1, :],
                        )

    def pipeline(unroll):
        # Software pipeline with ~1.5-step lookahead: every PE op of the
        # softmax chain for item i is emitted behind GEMM work of item i+1,
        # so its cross-engine dependency (DVE recip / gate_n, ACT exp/cast)
        # has already resolved when the PE reaches it. `unroll` reps are
        # chained into ONE pipeline so the For_i barrier bubble is paid only
        # once per `unroll` reps.
        items = [
            (r, l, t) for r in range(unroll) for l in range(L) for t in range(NBT)
        ]
        n = len(items)
        ph_logits(items[0])
        ph_h(items[0], 0)
        ph_h(items[0], 1)
        ph_sum(items[0])
        for i in range(n):
            if i + 1 < n:
                ph_logits(items[i + 1])
                ph_r4(items[i])
                ph_h(items[i + 1], 0)
                ph_gbc(items[i])
                ph_h(items[i + 1], 1)
                ph_sum(items[i + 1])
                ph_mix(items[i])
            else:
                ph_r4(items[i])
                ph_gbc(items[i])
                ph_mix(items[i])

    if reps == 1:
        pipeline(1)
    else:
        # largest unroll dividing reps: the For_i all-engine barrier costs
        # ~16-25us per trip, so amortize it over as many invocations as the
        # instruction budget allows.
        unroll = next((u for u in (16, 8, 4, 2) if reps % u == 0), 1)
        with tc.For_i(0, reps // unroll, 1):
            pipeline(unroll)

    ctx.close()


def prep_inputs(x0, U, V, gW, gb, bias):
    """Host-side shard + transpose + param reshape. Returns list of per-core
    input dicts."""
    import ml_dtypes

    wnp = ml_dtypes.bfloat16 if MM_DTYPE == "bf16" else np.float32

    x0 = np.ascontiguousarray(np.asarray(x0, dtype=np.float32))
    U = np.asarray(U, dtype=np.float32)
    V = np.asarray(V, dtype=np.float32)
    gW = np.ascontiguousarray(np.asarray(gW, dtype=np.float32))
    gb = np.asarray(gb, dtype=np.float32)
    bias = np.asarray(bias, dtype=np.float32)

    U_all = np.ascontiguousarray(U.transpose(0, 2, 1, 3).reshape(L, D, ER))
    V_all = np.ascontiguousarray(V.reshape(L, ER, D))

    # B_l = sum_{j<=l} bias_j ; affine shifts folded into next-layer biases
    Bcum = np.cumsum(bias, axis=0)                     # [L, D], B_l
    Bprev = np.concatenate([np.zeros((1, D), np.float32), Bcum[:-1]], 0)  # B_{l-1}
    # exp bias: gb_l + B_{l-1} @ gW_l   -> [E, L]
    gbB = np.stack([gb[l] + Bprev[l] @ gW[l] for l in range(L)], axis=1)
    gbB = np.ascontiguousarray(gbB.astype(np.float32))
    # relu bias: B_{l-1} @ U_l  -> [L, ER] -> [128, L, EC]
    BU = np.stack([Bprev[l] @ U_all[l] for l in range(L)], axis=0)  # [L, ER]
    BU_t = np.ascontiguousarray(
        BU.reshape(L, EC, 128).transpose(2, 0, 1).astype(np.float32)
    )
    # final-layer bias: B_{L-1} [D] -> [128, DC]  (d = c*128 + p)
    B2_t = np.ascontiguousarray(Bcum[L - 1].reshape(DC, 128).T.astype(np.float32))

    sel = np.zeros((E, EC, 128), np.float32)
    for m in range(EC):
        for p in range(128):
            sel[2 * m + p // 64, m, p] = 1.0
    ones44 = np.ones((4, 4), np.float32)

    shared = dict(
        U_all=U_all.astype(wnp),
        V_all=V_all.astype(wnp),
        gW_all=gW.astype(wnp),
        sel_h=sel.astype(wnp),
        ones_h=ones44.astype(wnp),
        gbB_h=gbB,
        BU_h=BU_t,
        B2_h=B2_t,
    )
    in_maps = []
    for c in range(N_CORES):
        shard = x0[c * BC : (c + 1) * BC, :]                           # [BC, D]
        x0T = np.ascontiguousarray(shard.T).astype(wnp)                # [D, BC]
        m = dict(shared)
        m["x0T"] = x0T
        in_maps.append(m)
    return in_maps


_compiled = None


def _get_compiled():
    global _compiled
    if _compiled is None:
        import jax
        import numpy as _np
        from jax.sharding import Mesh, PartitionSpec, NamedSharding
        from jax.experimental.shard_map import shard_map
        import concourse.bacc as bacc
        import concourse.mybir as mybir
        from concourse import bass2jax
        from concourse.tile import TileContext

        nc = bacc.Bacc(
            "TRN2", target_bir_lowering=False, debug=False, num_devices=N_CORES
        )
        with TileContext(nc) as tc:
            build_body(nc, tc, reps=1)
        nc.compile()

        in_names, out_names, out_avals = [], [], []
        zero_shapes = []
        for alloc in nc.m.functions[0].allocations:
            if not isinstance(alloc, mybir.MemoryLocationSet):
                continue
            name = alloc.memorylocations[0].name
            if alloc.kind == "ExternalInput":
                in_names.append(name)
            elif alloc.kind == "ExternalOutput":
                out_names.append(name)
                shape = tuple(alloc.tensor_shape)
                dtype = mybir.dt.np(alloc.dtype)
                out_avals.append(jax.core.ShapedArray(shape, dtype))
                zero_shapes.append((shape, dtype))

        def _bass_body(*args):
            outs = bass2jax._bass_exec_p.bind(
                *args,
                out_avals=tuple(out_avals),
                in_names=tuple(in_names + out_names),
                out_names=tuple(out_names),
                lowering_input_output_aliases=(),
                sim_require_finite=True,
                sim_require_nnan=True,
                nc=nc,
            )
            return tuple(outs)

        devices = jax.devices()[:N_CORES]
        mesh = Mesh(_np.asarray(devices), ("core",))
        n_params, n_outs = len(in_names), len(out_names)
        fn = jax.jit(
            shard_map(
                _bass_body,
                mesh=mesh,
                in_specs=(PartitionSpec("core"),) * (n_params + n_outs),
                out_specs=(PartitionSpec("core"),) * n_outs,
                check_rep=False,
            ),
            keep_unused=True,
        )
        sharding = NamedSharding(mesh, PartitionSpec("core"))
        _compiled = (fn, in_names, out_names, out_avals, zero_shapes, sharding)
    return _compiled


def run_device(in_maps):
    """Run the compiled NEFF on 8 cores; returns list of per-core out dicts."""
    import jax

    fn, in_names, out_names, out_avals, zero_shapes, sharding = _get_compiled()
    concat = []
    for n in in_names:
        if n == "partition_id":
            concat.append(
                np.arange(N_CORES, dtype=np.uint32).reshape(N_CORES, 1)
            )
        else:
            concat.append(
                np.concatenate([np.asarray(m[n]) for m in in_maps], axis=0)
            )
    zeros = [np.zeros((N_CORES * s[0], *s[1:]), d) for (s, d) in zero_shapes]
    dev_args = [jax.device_put(a, sharding) for a in concat + zeros]
    outs = fn(*dev_args)
    jax.block_until_ready(outs)
    res = []
    for c in range(N_CORES):
        d = {}
        for i, name in enumerate(out_names):
            shape = out_avals[i].shape
            d[name] = np.asarray(outs[i]).reshape(N_CORES, *shape)[c]
        res.append(d)
    return res


def kernel(x0, U, V, gW, gb, bias):
    in_maps = prep_inputs(x0, U, V, gW, gb, bias)
    res = run_device(in_maps)
    out = np.empty((B, D), np.float32)
    for c in range(N_CORES):
        out[c * BC : (c + 1) * BC, :] = res[c]["yT"].T
    return out



# revision 4
# speedup vs baseline: 1.0777x; 1.0777x over previous
"""CrossNetV2 soft-MoE kernel for 8 axon-tunneled TRN2 NeuronCores (v2).

Problem (hardcoded shapes): B=16384, D=1024, R=64, E=4, L=3.
    for l in range(L):
        h         = relu(einsum('bd,edr->ber', x, U[l]))          # [B,E,R]
        expert    = einsum('ber,erd->bed', h, V[l])               # [B,E,D]
        gate      = softmax(x @ gW[l] + gb[l])                    # [B,E]
        mixed     = einsum('be,bed->bd', gate, expert)            # [B,D]
        x         = x0 * mixed + bias[l] + x

Strategy (v2):
  - Data parallel: batch split 8 ways (2048 rows/core); params replicated.
  - Feature-on-partitions layout; host transposes x0 to [D, B_core].
  - Running-sum reformulation: x_l = x0*M_l + B_l, M_l = 1 + sum_{j<l} mixed_j,
    B_l = cumsum(bias) (const).  B_l folded into next-layer matmul biases
    (relu bias B@U, exp bias B@gW + gb); the FINAL B_2 is added on the host.
  - Packed gating: gW zero-padded to 32 columns; all 4 batch-tiles' logits
    land in ONE PSUM bank at partition offsets 0/32/64/96 (tile_position), so
    exp / sum / recip / cast / r4-broadcast / gate_n each run ONCE per layer
    instead of once per tile.  Sum and r4 use one-hot selector matmuls.
  - M kept in bf16 (error ~0.3% of M, well within budget): xb = M*x0 is an
    all-bf16 mul (2x DVE rate, GpSimd-eligible since it never touches PSUM).
  - Last layer: M preloaded into PSUM banks (ACT), V-matmuls accumulate on
    top; y = bf16(M_f * x0) DMA'd out; host adds B_2 and upcasts to f32.
  - Engine balance per rep: PE ~108us (510 matmuls), DVE ~60, ACT ~55,
    Pool ~40 (xb bulk), DMA ~23 (bf16 y halves output traffic).
"""

import os
import sys

sys.path.insert(0, "/opt/trn_rl_repo")

import numpy as np

B, D, R, E, L = 16384, 1024, 64, 4, 3
N_CORES = 8
BC = B // N_CORES          # rows per core
BT = 512                   # batch-tile (free dim / PSUM bank width)
NBT = BC // BT             # b-tiles per core
ER = E * R                 # 256
DC = D // 128              # 8 d-chunks
EC = ER // 128             # 2 er-chunks
GP = 32                    # gW padded width (tile_position block)

# xb pair-ops (of 4 per tile-layer) assigned to GpSimd; rest on DVE
XB_POOL = int(os.environ.get("KXB_POOL", "3"))


def build_body(nc, tc, reps=1):
    import concourse.mybir as mybir

    f32 = mybir.dt.float32
    bf16 = mybir.dt.bfloat16
    AF = mybir.ActivationFunctionType

    x0T = nc.dram_tensor("x0T", [D, BC], bf16, kind="ExternalInput")
    U_all = nc.dram_tensor("U_all", [L, D, ER], bf16, kind="ExternalInput")
    V_all = nc.dram_tensor("V_all", [L, ER, D], bf16, kind="ExternalInput")
    gWp_all = nc.dram_tensor("gWp_all", [L, D, GP], bf16, kind="ExternalInput")
    sumsel_h = nc.dram_tensor("sumsel_h", [128, NBT], bf16, kind="ExternalInput")
    r4sel_h = nc.dram_tensor("r4sel_h", [NBT, 128], bf16, kind="ExternalInput")
    seltm_h = nc.dram_tensor("seltm_h", [128, NBT * EC, 128], bf16,
                             kind="ExternalInput")
    gbB_h = nc.dram_tensor("gbB_h", [128, L], f32, kind="ExternalInput")
    BU_h = nc.dram_tensor("BU_h", [128, L, EC], f32, kind="ExternalInput")
    yT = nc.dram_tensor("yT", [D, BC], bf16, kind="ExternalOutput")

    from contextlib import ExitStack

    ctx = ExitStack()
    const = ctx.enter_context(tc.tile_pool(name="const", bufs=1))
    x0p = ctx.enter_context(tc.tile_pool(name="x0p", bufs=NBT))
    Mp = ctx.enter_context(tc.tile_pool(name="Mp", bufs=NBT))
    xbp = ctx.enter_context(tc.tile_pool(name="xbp", bufs=NBT))
    explogp = ctx.enter_context(tc.tile_pool(name="explogp", bufs=2))
    gatenp = ctx.enter_context(tc.tile_pool(name="gatenp", bufs=2))
    smallp = ctx.enter_context(tc.tile_pool(name="smallp", bufs=2))
    hrp = ctx.enter_context(tc.tile_pool(name="hrp", bufs=6))
    hsp = ctx.enter_context(tc.tile_pool(name="hsp", bufs=3))
    tdp = ctx.enter_context(tc.tile_pool(name="tdp", bufs=4))
    ps_small = ctx.enter_context(tc.tile_pool(name="ps_small", bufs=1,
                                              space="PSUM"))
    ps_h = ctx.enter_context(tc.tile_pool(name="ps_h", bufs=2, space="PSUM"))
    ps_gbc = ctx.enter_context(tc.tile_pool(name="ps_gbc", bufs=2, space="PSUM"))
    ps_mx = ctx.enter_context(tc.tile_pool(name="ps_mx", bufs=3, space="PSUM"))

    # ---- constants (loaded once, outside the reps loop) ----
    Ur = const.tile([128, L, DC, ER], bf16)
    Vr = const.tile([128, L, EC, D], bf16)
    gWpr = const.tile([128, L, DC, GP], bf16)
    sumsel = const.tile([128, NBT], bf16)
    r4sel = const.tile([NBT, 128], bf16)
    seltm = const.tile([128, NBT * EC, 128], bf16)
    gbBt = const.tile([128, L], f32)
    BUr = const.tile([128, L, EC], f32)

    nc.sync.dma_start(out=gWpr,
                      in_=gWp_all.rearrange("l (c p) e -> p l c e", p=128))
    nc.sync.dma_start(out=sumsel, in_=sumsel_h[:, :])
    nc.sync.dma_start(out=r4sel, in_=r4sel_h[:, :])
    nc.sync.dma_start(out=seltm, in_=seltm_h[:, :, :])
    nc.sync.dma_start(out=gbBt, in_=gbB_h[:, :])
    nc.sync.dma_start(out=BUr, in_=BU_h[:, :, :])
    for l in range(L):
        nc.sync.dma_start(
            out=Ur[:, l],
            in_=U_all[l].rearrange("(c p) er -> p c er", p=128),
        )
        nc.sync.dma_start(
            out=Vr[:, l],
            in_=V_all[l].rearrange("(k p) d -> p k d", p=128),
        )

    # persistent per-tile state
    x0_t = [x0p.tile([128, DC, BT], bf16, tag="x0", name=f"x0_{i}")
            for i in range(NBT)]
    M_t = [Mp.tile([128, DC, BT], bf16, tag="M", name=f"M_{i}")
           for i in range(NBT)]
    xb_t = [xbp.tile([128, DC, BT], bf16, tag="xb", name=f"xb_{i}")
            for i in range(NBT)]

    state = {}

    def ph_logits(key, t):
        r, l = key
        if l == 0:
            b0 = t * BT
            nc.sync.dma_start(
                out=x0_t[t],
                in_=x0T[:, b0: b0 + BT].rearrange("(c p) b -> p c b", p=128),
            )
        xin = x0_t[t] if l == 0 else xb_t[t]
        if t == 0:
            lg = ps_small.tile([128, BT], f32, tag="sm", name="lg")
            state[key] = {"lg": lg, "hr": {}, "hs": {}}
        lg = state[key]["lg"]
        for c in range(DC):
            nc.tensor.matmul(
                lg[32 * t: 32 * t + 32, :],
                gWpr[:, l, c, :],
                xin[:, c, :],
                start=(c == 0),
                stop=(c == DC - 1),
                tile_position=(0, 32 * t),
            )

    def ph_exp(key):
        r, l = key
        st = state[key]
        explog = explogp.tile([128, BT], bf16, tag="explog")
        nc.scalar.activation(out=explog, in_=st["lg"], func=AF.Exp,
                             bias=gbBt[:, l: l + 1], scale=1.0)
        st["explog"] = explog

    def ph_sum(key):
        st = state[key]
        sa = ps_small.tile([NBT, BT], f32, tag="sm")
        nc.tensor.matmul(sa, sumsel[:, :], st["explog"], start=True, stop=True)
        recip = smallp.tile([NBT, BT], f32, tag="recip")
        nc.vector.reciprocal_approx_fast(out=recip, in_=sa)
        recipb = smallp.tile([NBT, BT], bf16, tag="recipb")
        nc.scalar.copy(out=recipb, in_=recip)
        st["recipb"] = recipb

    def ph_r4(key):
        st = state[key]
        r4 = ps_small.tile([128, BT], f32, tag="sm")
        nc.tensor.matmul(r4, r4sel[:, :], st["recipb"], start=True, stop=True)
        gaten = gatenp.tile([128, BT], bf16, tag="gaten")
        with nc.allow_low_precision(reason="gate in matmul dtype"):
            nc.vector.tensor_mul(out=gaten, in0=st["explog"], in1=r4)
        st["gaten"] = gaten

    def ph_U(key, t, m):
        r, l = key
        xin = x0_t[t] if l == 0 else xb_t[t]
        hm = ps_h.tile([128, BT], f32, tag="hp")
        for c in range(DC):
            nc.tensor.matmul(
                hm,
                Ur[:, l, c, m * 128: (m + 1) * 128],
                xin[:, c, :],
                start=(c == 0),
                stop=(c == DC - 1),
            )
        hr = hrp.tile([128, BT], bf16, tag="hr")
        nc.scalar.activation(out=hr, in_=hm, func=AF.Relu,
                             bias=BUr[:, l, m: m + 1], scale=1.0)
        state[key]["hr"][(t, m)] = hr

    def ph_gbc(key, t):
        st = state[key]
        hs = hsp.tile([128, EC, BT], bf16, tag="hs")
        for m in range(EC):
            gb = ps_gbc.tile([128, BT], f32, tag="gbc")
            nc.tensor.matmul(gb, seltm[:, EC * t + m, :], st["gaten"],
                             start=True, stop=True)
            with nc.allow_low_precision(reason="hs in matmul dtype"):
                nc.vector.tensor_mul(out=hs[:, m, :], in0=st["hr"].pop((t, m)),
                                     in1=gb)
        st["hs"][t] = hs

    def ph_V(key, t):
        r, l = key
        b0 = t * BT
        st = state[key]
        hs = st["hs"].pop(t)
        last = l == L - 1
        for d in range(DC):
            mx = ps_mx.tile([128, BT], f32, tag="mx")
            if last:
                # preload M into the PSUM bank; V-matmuls accumulate on top.
                nc.scalar.copy(out=mx, in_=M_t[t][:, d, :])
            for k in range(EC):
                nc.tensor.matmul(
                    mx,
                    Vr[:, l, k, d * 128: (d + 1) * 128],
                    hs[:, k, :],
                    start=(k == 0 and not last),
                    stop=(k == EC - 1),
                    skip_group_check=last,
                )
            if last:
                td = tdp.tile([128, BT], bf16, tag="td")
                with nc.allow_low_precision(reason="y output in bf16"):
                    nc.vector.tensor_mul(out=td, in0=mx, in1=x0_t[t][:, d, :])
                nc.sync.dma_start(
                    out=yT[:, b0: b0 + BT].rearrange("(c p) b -> p c b",
                                                     p=128)[:, d, :],
                    in_=td,
                )
            else:
                if l == 0:
                    # M = mixed + 1  (bf16)
                    with nc.allow_low_precision(reason="M in bf16"):
                        nc.scalar.activation(
                            out=M_t[t][:, d, :], in_=mx, func=AF.Identity,
                            bias=1.0, scale=1.0,
                        )
                else:
                    with nc.allow_low_precision(reason="M in bf16"):
                        nc.vector.tensor_add(
                            out=M_t[t][:, d, :], in0=M_t[t][:, d, :], in1=mx
                        )
                if d % 2 == 1:
                    # all-bf16 pair mul; pair index (d-1)//2 in 0..3
                    eng = nc.gpsimd if (d - 1) // 2 < XB_POOL else nc.vector
                    with nc.allow_low_precision(reason="xb in matmul dtype"):
                        eng.tensor_mul(
                            out=xb_t[t][:, d - 1: d + 1, :],
                            in0=M_t[t][:, d - 1: d + 1, :],
                            in1=x0_t[t][:, d - 1: d + 1, :],
                        )

    def emit_step(key):
        # Emit order interleaves the cross-engine gating chain behind PE
        # GEMM work so each dependency has resolved when its consumer engine
        # reaches it.
        for t in range(NBT):
            ph_logits(key, t)
        ph_exp(key)
        ph_U(key, 0, 0)
        ph_U(key, 0, 1)
        ph_sum(key)
        ph_U(key, 1, 0)
        ph_U(key, 1, 1)
        ph_r4(key)
        ph_U(key, 2, 0)
        ph_U(key, 2, 1)
        ph_gbc(key, 0)
        ph_gbc(key, 1)
        ph_U(key, 3, 0)
        ph_U(key, 3, 1)
        ph_gbc(key, 2)
        ph_gbc(key, 3)
        for t in range(NBT):
            ph_V(key, t)
        state.pop(key)

    def pipeline(unroll):
        for r in range(unroll):
            for l in range(L):
                emit_step((r, l))

    if reps == 1:
        pipeline(1)
    elif os.environ.get("KFORCE_UNROLL"):
        pipeline(reps)
    else:
        unroll = next((u for u in (16, 8, 4, 2) if reps % u == 0), 1)
        with tc.For_i(0, reps // unroll, 1):
            pipeline(unroll)

    ctx.close()


def prep_inputs(x0, U, V, gW, gb, bias):
    """Host-side shard + transpose + param reshape. Returns list of per-core
    input dicts."""
    import ml_dtypes

    bf = ml_dtypes.bfloat16

    x0 = np.ascontiguousarray(np.asarray(x0, dtype=np.float32))
    U = np.asarray(U, dtype=np.float32)
    V = np.asarray(V, dtype=np.float32)
    gW = np.ascontiguousarray(np.asarray(gW, dtype=np.float32))
    gb = np.asarray(gb, dtype=np.float32)
    bias = np.asarray(bias, dtype=np.float32)

    U_all = np.ascontiguousarray(U.transpose(0, 2, 1, 3).reshape(L, D, ER))
    V_all = np.ascontiguousarray(V.reshape(L, ER, D))

    # B_l = sum_{j<=l} bias_j ; affine shifts folded into next-layer biases
    Bcum = np.cumsum(bias, axis=0)                     # [L, D], B_l
    Bprev = np.concatenate([np.zeros((1, D), np.float32), Bcum[:-1]], 0)

    # gW zero-padded to GP columns
    gWp = np.zeros((L, D, GP), np.float32)
    gWp[:, :, :E] = gW

    # exp bias rows 32t+e <- gb[l,e] + Bprev[l] @ gW[l][:,e]; rest 0
    gbB = np.stack([gb[l] + Bprev[l] @ gW[l] for l in range(L)], axis=1)  # [E,L]
    gbB_all = np.zeros((128, L), np.float32)
    for t in range(NBT):
        gbB_all[32 * t: 32 * t + E, :] = gbB

    # relu bias: B_{l-1} @ U_l  -> [L, ER] -> [128, L, EC]
    BU = np.stack([Bprev[l] @ U_all[l] for l in range(L)], axis=0)  # [L, ER]
    BU_t = np.ascontiguousarray(
        BU.reshape(L, EC, 128).transpose(2, 0, 1).astype(np.float32)
    )

    # selectors
    sumsel = np.zeros((128, NBT), np.float32)
    for t in range(NBT):
        for e in range(E):
            sumsel[32 * t + e, t] = 1.0
    r4sel = np.zeros((NBT, 128), np.float32)
    for p in range(128):
        r4sel[p // 32, p] = 1.0
    seltm = np.zeros((128, NBT * EC, 128), np.float32)
    for t in range(NBT):
        for m in range(EC):
            for p in range(128):
                seltm[32 * t + 2 * m + p // 64, EC * t + m, p] = 1.0

    shared = dict(
        U_all=U_all.astype(bf),
        V_all=V_all.astype(bf),
        gWp_all=gWp.astype(bf),
        sumsel_h=sumsel.astype(bf),
        r4sel_h=r4sel.astype(bf),
        seltm_h=seltm.astype(bf),
        gbB_h=gbB_all,
        BU_h=BU_t,
    )
    in_maps = []
    for c in range(N_CORES):
        shard = x0[c * BC: (c + 1) * BC, :]                           # [BC, D]
        x0Tc = np.ascontiguousarray(shard.T).astype(bf)               # [D, BC]
        m = dict(shared)
        m["x0T"] = x0Tc
        in_maps.append(m)
    return in_maps


_compiled = None


def _get_compiled():
    global _compiled
    if _compiled is None:
        import jax
        import numpy as _np
        from jax.sharding import Mesh, PartitionSpec, NamedSharding
        from jax.experimental.shard_map import shard_map
        import concourse.bacc as bacc
        import concourse.mybir as mybir
        from concourse import bass2jax
        from concourse.tile import TileContext

        nc = bacc.Bacc(
            "TRN2", target_bir_lowering=False, debug=False, num_devices=N_CORES
        )
        with TileContext(nc) as tc:
            build_body(nc, tc, reps=1)
        nc.compile()

        in_names, out_names, out_avals = [], [], []
        zero_shapes = []
        for alloc in nc.m.functions[0].allocations:
            if not isinstance(alloc, mybir.MemoryLocationSet):
                continue
            name = alloc.memorylocations[0].name
            if alloc.kind == "ExternalInput":
                in_names.append(name)
            elif alloc.kind == "ExternalOutput":
                out_names.append(name)
                shape = tuple(alloc.tensor_shape)
                dtype = mybir.dt.np(alloc.dtype)
                out_avals.append(jax.core.ShapedArray(shape, dtype))
                zero_shapes.append((shape, dtype))

        def _bass_body(*args):
            outs = bass2jax._bass_exec_p.bind(
                *args,
                out_avals=tuple(out_avals),
                in_names=tuple(in_names + out_names),
                out_names=tuple(out_names),
                lowering_input_output_aliases=(),
                sim_require_finite=True,
                sim_require_nnan=True,
                nc=nc,
            )
            return tuple(outs)

        devices = jax.devices()[:N_CORES]
        mesh = Mesh(_np.asarray(devices), ("core",))
        n_params, n_outs = len(in_names), len(out_names)
        fn = jax.jit(
            shard_map(
                _bass_body,
                mesh=mesh,
                in_specs=(PartitionSpec("core"),) * (n_params + n_outs),
                out_specs=(PartitionSpec("core"),) * n_outs,
                check_rep=False,
            ),
            keep_unused=True,
        )
        sharding = NamedSharding(mesh, PartitionSpec("core"))
        _compiled = (fn, in_names, out_names, out_avals, zero_shapes, sharding)
    return _compiled


def run_device(in_maps):
    """Run the compiled NEFF on 8 cores; returns list of per-core out dicts."""
    import jax

    fn, in_names, out_names, out_avals, zero_shapes, sharding = _get_compiled()
    concat = []
    for n in in_names:
        if n == "partition_id":
            concat.append(
                np.arange(N_CORES, dtype=np.uint32).reshape(N_CORES, 1)
            )
        else:
            concat.append(
                np.concatenate([np.asarray(m[n]) for m in in_maps], axis=0)
            )
    zeros = [np.zeros((N_CORES * s[0], *s[1:]), d) for (s, d) in zero_shapes]
    dev_args = [jax.device_put(a, sharding) for a in concat + zeros]
    outs = fn(*dev_args)
    jax.block_until_ready(outs)
    res = []
    for c in range(N_CORES):
        d = {}
        for i, name in enumerate(out_names):
            shape = out_avals[i].shape
            d[name] = np.asarray(outs[i]).reshape(N_CORES, *shape)[c]
        res.append(d)
    return res


def kernel(x0, U, V, gW, gb, bias):
    in_maps = prep_inputs(x0, U, V, gW, gb, bias)
    res = run_device(in_maps)
    bias = np.asarray(bias, dtype=np.float32)
    B2 = np.cumsum(bias, axis=0)[L - 1]                 # final-layer bias [D]
    out = np.empty((B, D), np.float32)
    for c in range(N_CORES):
        out[c * BC: (c + 1) * BC, :] = (
            res[c]["yT"].astype(np.float32).T + B2[None, :]
        )
    return out


# revision 9
# speedup vs baseline: 1.1417x; 1.0594x over previous
"""CrossNetV2 soft-MoE kernel for 8 axon-tunneled TRN2 NeuronCores (v2).

Problem (hardcoded shapes): B=16384, D=1024, R=64, E=4, L=3.
    for l in range(L):
        h         = relu(einsum('bd,edr->ber', x, U[l]))          # [B,E,R]
        expert    = einsum('ber,erd->bed', h, V[l])               # [B,E,D]
        gate      = softmax(x @ gW[l] + gb[l])                    # [B,E]
        mixed     = einsum('be,bed->bd', gate, expert)            # [B,D]
        x         = x0 * mixed + bias[l] + x

Strategy (v3):
  - Data parallel: batch split 8 ways (2048 rows/core); params replicated.
  - Feature-on-partitions layout; host transposes x0 to [D, B_core].
  - Running-sum reformulation: x_l = x0*M_l + B_l, M_l = 1 + sum_{j<l} mixed_j,
    B_l = cumsum(bias) (const).  B_l folded into next-layer matmul biases
    (relu bias B@U, exp bias B@gW + gb); the FINAL B_2 is added on the host.
  - Packed gating: gW zero-padded to 32 columns; all 4 batch-tiles' logits
    land in ONE PSUM bank at partition offsets 0/32/64/96 (tile_position), so
    exp / sum / recip / cast / r4-broadcast / gate_n each run ONCE per layer
    instead of once per tile.  Sum and r4 use one-hot selector matmuls.
  - NO running-M tensor: the xb-chain identity  xb_{l+1} = xb_l + bf16(mix_l
    * x0)  (xb_1 = x0 + t_0, y = xb_2 + t_2) folds the running sum into the
    bf16 matmul-input tensor itself.  Every mixed element is touched exactly
    twice: one PSUM-side mul (t = mix*x0) and one bf16 add.
  - PSUM mixed banks are PAIR tiles [128,2,BT] (2 banks); drains alternate
    DVE-direct (t = mix*x0, PSUM port) and ACT-copy (mixsc = bf16(mix), bank
    freed, mul done later on DVE/Pool at bf16 rate) so banks free at ~2x the
    single-engine rate and the PE's V-phase never waits.
  - bf16 adds (xb update / y) are deferred and split DVE/GpSimd.
  - x0 for rep r+1 prefetched on the ACT DMA queue during rep r's layer 2
    (y-writes stay on the SP queue, so loads never queue behind stores).
  - Engine balance per rep: PE ~108us sim (510 matmuls, the dataflow minimum
    given 128-wide PE output chunks), DVE ~70, ACT ~48, Pool ~30, DMA ~23.
"""

import os
import sys

sys.path.insert(0, "/opt/trn_rl_repo")

import numpy as np

B, D, R, E, L = 16384, 1024, 64, 4, 3
N_CORES = 8
BC = B // N_CORES          # rows per core
BT = 512                   # batch-tile (free dim / PSUM bank width)
NBT = BC // BT             # b-tiles per core
ER = E * R                 # 256
DC = D // 128              # 8 d-chunks
EC = ER // 128             # 2 er-chunks
GP = 32                    # gW padded width (tile_position block)

# tiles whose deferred xb-adds ride GpSimd (early tiles have latency slack);
# the remaining tiles' adds use the faster DVE
ADD_POOL_TILES = int(os.environ.get("KADD_POOL_TILES", "2"))


def build_body(nc, tc, reps=1):
    import concourse.mybir as mybir

    f32 = mybir.dt.float32
    bf16 = mybir.dt.bfloat16
    AF = mybir.ActivationFunctionType

    x0T = nc.dram_tensor("x0T", [D, BC], bf16, kind="ExternalInput")
    U_all = nc.dram_tensor("U_all", [L, D, ER], bf16, kind="ExternalInput")
    V_all = nc.dram_tensor("V_all", [L, ER, D], bf16, kind="ExternalInput")
    gWp_all = nc.dram_tensor("gWp_all", [L, D, GP], bf16, kind="ExternalInput")
    sumsel_h = nc.dram_tensor("sumsel_h", [128, NBT], bf16, kind="ExternalInput")
    r4sel_h = nc.dram_tensor("r4sel_h", [NBT, 128], bf16, kind="ExternalInput")
    seltm_h = nc.dram_tensor("seltm_h", [128, NBT * EC, 128], bf16,
                             kind="ExternalInput")
    gbB_h = nc.dram_tensor("gbB_h", [128, L], f32, kind="ExternalInput")
    BU_h = nc.dram_tensor("BU_h", [128, L, EC], f32, kind="ExternalInput")
    yT = nc.dram_tensor("yT", [D, BC], bf16, kind="ExternalOutput")

    from contextlib import ExitStack

    ctx = ExitStack()
    const = ctx.enter_context(tc.tile_pool(name="const", bufs=1))
    x0p = ctx.enter_context(tc.tile_pool(name="x0p", bufs=NBT))
    xbp = ctx.enter_context(tc.tile_pool(name="xbp", bufs=NBT))
    explogp = ctx.enter_context(tc.tile_pool(name="explogp", bufs=2))
    gatenp = ctx.enter_context(tc.tile_pool(name="gatenp", bufs=2))
    smallp = ctx.enter_context(tc.tile_pool(name="smallp", bufs=2))
    hrp = ctx.enter_context(tc.tile_pool(name="hrp", bufs=6))
    hsp = ctx.enter_context(tc.tile_pool(name="hsp", bufs=3))
    tpp = ctx.enter_context(tc.tile_pool(name="tpp", bufs=4))
    mscp = ctx.enter_context(tc.tile_pool(name="mscp", bufs=3))
    tdp = ctx.enter_context(tc.tile_pool(name="tdp", bufs=3))
    ps_small = ctx.enter_context(tc.tile_pool(name="ps_small", bufs=1,
                                              space="PSUM"))
    # h-banks and gate-broadcast banks share one rotating pool
    ps_h = ctx.enter_context(tc.tile_pool(name="ps_h", bufs=3, space="PSUM"))
    # mixed single banks; drains alternate DVE/ACT per bank
    ps_mx = ctx.enter_context(tc.tile_pool(name="ps_mx", bufs=4, space="PSUM"))

    # ---- constants (loaded once, outside the reps loop) ----
    Ur = const.tile([128, L, DC, ER], bf16)
    Vr = const.tile([128, L, EC, D], bf16)
    gWpr = const.tile([128, L, DC, GP], bf16)
    sumsel = const.tile([128, NBT], bf16)
    r4sel = const.tile([NBT, 128], bf16)
    seltm = const.tile([128, NBT * EC, 128], bf16)
    gbBt = const.tile([128, L], f32)
    BUr = const.tile([128, L, EC], f32)

    nc.sync.dma_start(out=gWpr,
                      in_=gWp_all.rearrange("l (c p) e -> p l c e", p=128))
    nc.sync.dma_start(out=sumsel, in_=sumsel_h[:, :])
    nc.sync.dma_start(out=r4sel, in_=r4sel_h[:, :])
    nc.sync.dma_start(out=seltm, in_=seltm_h[:, :, :])
    nc.sync.dma_start(out=gbBt, in_=gbB_h[:, :])
    nc.sync.dma_start(out=BUr, in_=BU_h[:, :, :])
    for l in range(L):
        nc.sync.dma_start(
            out=Ur[:, l],
            in_=U_all[l].rearrange("(c p) er -> p c er", p=128),
        )
        nc.sync.dma_start(
            out=Vr[:, l],
            in_=V_all[l].rearrange("(k p) d -> p k d", p=128),
        )

    # persistent per-tile state
    x0_t = [x0p.tile([128, DC, BT], bf16, tag="x0", name=f"x0_{i}")
            for i in range(NBT)]
    xb_t = [xbp.tile([128, DC, BT], bf16, tag="xb", name=f"xb_{i}")
            for i in range(NBT)]

    state = {}

    def emit_x0_load(t):
        # Issued from the ACT queue so input loads never queue behind the
        # SP-queue y-writes.
        b0 = t * BT
        nc.scalar.dma_start(
            out=x0_t[t],
            in_=x0T[:, b0: b0 + BT].rearrange("(c p) b -> p c b", p=128),
        )

    def ph_logits(key, t):
        r, l = key
        xin = x0_t[t] if l == 0 else xb_t[t]
        if t == 0:
            lg = ps_small.tile([128, BT], f32, tag="sm", name="lg")
            state[key] = {"lg": lg, "hr": {}, "hs": {}}
        lg = state[key]["lg"]
        for c in range(DC):
            nc.tensor.matmul(
                lg[32 * t: 32 * t + 32, :],
                gWpr[:, l, c, :],
                xin[:, c, :],
                start=(c == 0),
                stop=(c == DC - 1),
                tile_position=(0, 32 * t),
            )

    def ph_exp(key):
        r, l = key
        st = state[key]
        explog = explogp.tile([128, BT], bf16, tag="explog")
        nc.scalar.activation(out=explog, in_=st["lg"], func=AF.Exp,
                             bias=gbBt[:, l: l + 1], scale=1.0)
        st["explog"] = explog

    def ph_sum(key):
        st = state[key]
        sa = ps_small.tile([NBT, BT], f32, tag="sm")
        nc.tensor.matmul(sa, sumsel[:, :], st["explog"], start=True, stop=True)
        recip = smallp.tile([NBT, BT], f32, tag="recip")
        nc.vector.reciprocal_approx_fast(out=recip, in_=sa)
        recipb = smallp.tile([NBT, BT], bf16, tag="recipb")
        nc.scalar.copy(out=recipb, in_=recip)
        st["recipb"] = recipb

    def ph_r4(key):
        st = state[key]
        r4 = ps_small.tile([128, BT], f32, tag="sm")
        nc.tensor.matmul(r4, r4sel[:, :], st["recipb"], start=True, stop=True)
        gaten = gatenp.tile([128, BT], bf16, tag="gaten")
        with nc.allow_low_precision(reason="gate in matmul dtype"):
            nc.vector.tensor_mul(out=gaten, in0=st["explog"], in1=r4)
        st["gaten"] = gaten

    def ph_U(key, t, m):
        r, l = key
        xin = x0_t[t] if l == 0 else xb_t[t]
        hm = ps_h.tile([128, BT], f32, tag="hp")
        for c in range(DC):
            nc.tensor.matmul(
                hm,
                Ur[:, l, c, m * 128: (m + 1) * 128],
                xin[:, c, :],
                start=(c == 0),
                stop=(c == DC - 1),
            )
        hr = hrp.tile([128, BT], bf16, tag="hr")
        nc.scalar.activation(out=hr, in_=hm, func=AF.Relu,
                             bias=BUr[:, l, m: m + 1], scale=1.0)
        state[key]["hr"][(t, m)] = hr

    def ph_gbc(key, t):
        st = state[key]
        hs = hsp.tile([128, EC, BT], bf16, tag="hs")
        for m in range(EC):
            gb = ps_h.tile([128, BT], f32, tag="hp", name="gb")
            nc.tensor.matmul(gb, seltm[:, EC * t + m, :], st["gaten"],
                             start=True, stop=True)
            with nc.allow_low_precision(reason="hs in matmul dtype"):
                nc.vector.tensor_mul(out=hs[:, m, :], in0=st["hr"].pop((t, m)),
                                     in1=gb)
        st["hs"][t] = hs

    def ph_V_mm(key, t):
        # V matmuls into single banks + the bank-freeing drain ONLY.
        # Drains alternate DVE-direct / ACT-copy per bank so banks free at
        # ~2x the single-engine elementwise rate; outputs land in HALVES of
        # pair tiles so the deferred bf16 post-work stays pair-fused.
        r, l = key
        st = state[key]
        hs = st["hs"].pop(t)
        tprs = st.setdefault("tprs", {})
        for p in range(DC // 2):           # 4 pairs = 8 single-bank groups
            d0 = 2 * p
            if p % 2 == 0:
                buf = tpp.tile([128, 2, BT], bf16, tag="tp", name="tp")
                kind = "t"
            else:
                buf = mscp.tile([128, 2, BT], bf16, tag="msc", name="msc")
                kind = "m"
            tprs[(t, p)] = (kind, buf)
            for dd in range(2):
                mx = ps_mx.tile([128, BT], f32, tag="mx", name="mx")
                for k in range(EC):
                    nc.tensor.matmul(
                        mx,
                        Vr[:, l, k, (d0 + dd) * 128: (d0 + dd + 1) * 128],
                        hs[:, k, :],
                        start=(k == 0),
                        stop=(k == EC - 1),
                    )
                if kind == "t":
                    # DVE-direct drain: t = mix * x0 straight from PSUM
                    with nc.allow_low_precision(reason="t in bf16"):
                        nc.vector.tensor_mul(
                            out=buf[:, dd, :], in0=mx,
                            in1=x0_t[t][:, d0 + dd, :])
                else:
                    # ACT-copy drain: bank freed by the copy; mul deferred
                    nc.scalar.copy(out=buf[:, dd, :], in_=mx)

    def ph_V_post(key, t):
        # deferred bf16 muls (for ACT-copied pairs) + xb-chain adds
        r, l = key
        b0 = t * BT
        st = state[key]
        last = l == L - 1
        for p in range(DC // 2):
            d0 = 2 * p
            x0pr = x0_t[t][:, d0: d0 + 2, :]
            kind, buf = st["tprs"].pop((t, p))
            if kind == "m":
                tpr = tpp.tile([128, 2, BT], bf16, tag="tp", name="tp")
                with nc.allow_low_precision(reason="t in bf16"):
                    nc.vector.tensor_mul(out=tpr, in0=buf, in1=x0pr)
            else:
                tpr = buf
            # adds: early tiles ride the idle GpSimd queue; late tiles use
            # the faster DVE so next-step logits aren't kept waiting
            eng = nc.gpsimd if t < ADD_POOL_TILES else nc.vector
            xbpr = xb_t[t][:, d0: d0 + 2, :]
            with nc.allow_low_precision(reason="xb chain in bf16"):
                if l == 0:
                    eng.tensor_add(out=xbpr, in0=x0pr, in1=tpr)
                elif l == 1:
                    eng.tensor_add(out=xbpr, in0=xbpr, in1=tpr)
                else:
                    td = tdp.tile([128, 2, BT], bf16, tag="td", name="td")
                    eng.tensor_add(out=td, in0=xbpr, in1=tpr)
                    nc.sync.dma_start(
                        out=yT[:, b0: b0 + BT].rearrange(
                            "(c p) b -> p c b", p=128)[:, d0: d0 + 2, :],
                        in_=td,
                    )
        if last:
            # prefetch next rep's x0 tile (same data every rep; ordering
            # after this rep's last x0 read keeps the race detector happy)
            emit_x0_load(t)

    def emit_step(key):
        # Emit order interleaves the cross-engine gating chain behind PE
        # GEMM work so each dependency has resolved when its consumer engine
        # reaches it.
        for t in range(NBT):
            ph_logits(key, t)
        ph_exp(key)
        ph_U(key, 0, 0)
        ph_U(key, 0, 1)
        ph_sum(key)
        ph_U(key, 1, 0)
        ph_U(key, 1, 1)
        ph_r4(key)
        ph_U(key, 2, 0)
        ph_gbc(key, 0)
        ph_U(key, 2, 1)
        ph_gbc(key, 1)
        ph_U(key, 3, 0)
        ph_gbc(key, 2)
        ph_U(key, 3, 1)
        ph_gbc(key, 3)
        ph_V_mm(key, 0)
        ph_V_mm(key, 1)
        ph_V_post(key, 0)
        ph_V_mm(key, 2)
        ph_V_post(key, 1)
        ph_V_mm(key, 3)
        ph_V_post(key, 2)
        ph_V_post(key, 3)
        state.pop(key)

    def pipeline(unroll):
        for r in range(unroll):
            for l in range(L):
                emit_step((r, l))

    def preamble():
        for t in range(NBT):
            emit_x0_load(t)

    if reps == 1:
        preamble()
        pipeline(1)
    elif os.environ.get("KFORCE_UNROLL"):
        preamble()
        pipeline(reps)
    else:
        unroll = next((u for u in (16, 8, 4, 2) if reps % u == 0), 1)
        preamble()
        with tc.For_i(0, reps // unroll, 1):
            pipeline(unroll)

    ctx.close()


def prep_inputs(x0, U, V, gW, gb, bias):
    """Host-side shard + transpose + param reshape. Returns list of per-core
    input dicts."""
    import ml_dtypes

    bf = ml_dtypes.bfloat16

    x0 = np.ascontiguousarray(np.asarray(x0, dtype=np.float32))
    U = np.asarray(U, dtype=np.float32)
    V = np.asarray(V, dtype=np.float32)
    gW = np.ascontiguousarray(np.asarray(gW, dtype=np.float32))
    gb = np.asarray(gb, dtype=np.float32)
    bias = np.asarray(bias, dtype=np.float32)

    U_all = np.ascontiguousarray(U.transpose(0, 2, 1, 3).reshape(L, D, ER))
    V_all = np.ascontiguousarray(V.reshape(L, ER, D))

    # B_l = sum_{j<=l} bias_j ; affine shifts folded into next-layer biases
    Bcum = np.cumsum(bias, axis=0)                     # [L, D], B_l
    Bprev = np.concatenate([np.zeros((1, D), np.float32), Bcum[:-1]], 0)

    # gW zero-padded to GP columns
    gWp = np.zeros((L, D, GP), np.float32)
    gWp[:, :, :E] = gW

    # exp bias rows 32t+e <- gb[l,e] + Bprev[l] @ gW[l][:,e]; rest 0
    gbB = np.stack([gb[l] + Bprev[l] @ gW[l] for l in range(L)], axis=1)  # [E,L]
    gbB_all = np.zeros((128, L), np.float32)
    for t in range(NBT):
        gbB_all[32 * t: 32 * t + E, :] = gbB

    # relu bias: B_{l-1} @ U_l  -> [L, ER] -> [128, L, EC]
    BU = np.stack([Bprev[l] @ U_all[l] for l in range(L)], axis=0)  # [L, ER]
    BU_t = np.ascontiguousarray(
        BU.reshape(L, EC, 128).transpose(2, 0, 1).astype(np.float32)
    )

    # selectors
    sumsel = np.zeros((128, NBT), np.float32)
    for t in range(NBT):
        for e in range(E):
            sumsel[32 * t + e, t] = 1.0
    r4sel = np.zeros((NBT, 128), np.float32)
    for p in range(128):
        r4sel[p // 32, p] = 1.0
    seltm = np.zeros((128, NBT * EC, 128), np.float32)
    for t in range(NBT):
        for m in range(EC):
            for p in range(128):
                seltm[32 * t + 2 * m + p // 64, EC * t + m, p] = 1.0

    shared = dict(
        U_all=U_all.astype(bf),
        V_all=V_all.astype(bf),
        gWp_all=gWp.astype(bf),
        sumsel_h=sumsel.astype(bf),
        r4sel_h=r4sel.astype(bf),
        seltm_h=seltm.astype(bf),
        gbB_h=gbB_all,
        BU_h=BU_t,
    )
    in_maps = []
    for c in range(N_CORES):
        shard = x0[c * BC: (c + 1) * BC, :]                           # [BC, D]
        x0Tc = np.ascontiguousarray(shard.T).astype(bf)               # [D, BC]
        m = dict(shared)
        m["x0T"] = x0Tc
        in_maps.append(m)
    return in_maps


_compiled = None


def _get_compiled():
    global _compiled
    if _compiled is None:
        import jax
        import numpy as _np
        from jax.sharding import Mesh, PartitionSpec, NamedSharding
        from jax.experimental.shard_map import shard_map
        import concourse.bacc as bacc
        import concourse.mybir as mybir
        from concourse import bass2jax
        from concourse.tile import TileContext

        nc = bacc.Bacc(
            "TRN2", target_bir_lowering=False, debug=False, num_devices=N_CORES
        )
        with TileContext(nc) as tc:
            build_body(nc, tc, reps=1)
        nc.compile()

        in_names, out_names, out_avals = [], [], []
        zero_shapes = []
        for alloc in nc.m.functions[0].allocations:
            if not isinstance(alloc, mybir.MemoryLocationSet):
                continue
            name = alloc.memorylocations[0].name
            if alloc.kind == "ExternalInput":
                in_names.append(name)
            elif alloc.kind == "ExternalOutput":
                out_names.append(name)
                shape = tuple(alloc.tensor_shape)
                dtype = mybir.dt.np(alloc.dtype)
                out_avals.append(jax.core.ShapedArray(shape, dtype))
                zero_shapes.append((shape, dtype))

        def _bass_body(*args):
            outs = bass2jax._bass_exec_p.bind(
                *args,
                out_avals=tuple(out_avals),
                in_names=tuple(in_names + out_names),
                out_names=tuple(out_names),
                lowering_input_output_aliases=(),
                sim_require_finite=True,
                sim_require_nnan=True,
                nc=nc,
            )
            return tuple(outs)

        devices = jax.devices()[:N_CORES]
        mesh = Mesh(_np.asarray(devices), ("core",))
        n_params, n_outs = len(in_names), len(out_names)
        fn = jax.jit(
            shard_map(
                _bass_body,
                mesh=mesh,
                in_specs=(PartitionSpec("core"),) * (n_params + n_outs),
                out_specs=(PartitionSpec("core"),) * n_outs,
                check_rep=False,
            ),
            keep_unused=True,
        )
        sharding = NamedSharding(mesh, PartitionSpec("core"))
        _compiled = (fn, in_names, out_names, out_avals, zero_shapes, sharding)
    return _compiled


def run_device(in_maps):
    """Run the compiled NEFF on 8 cores; returns list of per-core out dicts."""
    import jax

    fn, in_names, out_names, out_avals, zero_shapes, sharding = _get_compiled()
    concat = []
    for n in in_names:
        if n == "partition_id":
            concat.append(
                np.arange(N_CORES, dtype=np.uint32).reshape(N_CORES, 1)
            )
        else:
            concat.append(
                np.concatenate([np.asarray(m[n]) for m in in_maps], axis=0)
            )
    zeros = [np.zeros((N_CORES * s[0], *s[1:]), d) for (s, d) in zero_shapes]
    dev_args = [jax.device_put(a, sharding) for a in concat + zeros]
    outs = fn(*dev_args)
    jax.block_until_ready(outs)
    res = []
    for c in range(N_CORES):
        d = {}
        for i, name in enumerate(out_names):
            shape = out_avals[i].shape
            d[name] = np.asarray(outs[i]).reshape(N_CORES, *shape)[c]
        res.append(d)
    return res


def kernel(x0, U, V, gW, gb, bias):
    in_maps = prep_inputs(x0, U, V, gW, gb, bias)
    res = run_device(in_maps)
    bias = np.asarray(bias, dtype=np.float32)
    B2 = np.cumsum(bias, axis=0)[L - 1]                 # final-layer bias [D]
    out = np.empty((B, D), np.float32)
    for c in range(N_CORES):
        out[c * BC: (c + 1) * BC, :] = (
            res[c]["yT"].astype(np.float32).T + B2[None, :]
        )
    return out


# revision 12
# speedup vs baseline: 1.2085x; 1.0586x over previous
"""CrossNetV2 soft-MoE kernel for 8 axon-tunneled TRN2 NeuronCores (v2).

Problem (hardcoded shapes): B=16384, D=1024, R=64, E=4, L=3.
    for l in range(L):
        h         = relu(einsum('bd,edr->ber', x, U[l]))          # [B,E,R]
        expert    = einsum('ber,erd->bed', h, V[l])               # [B,E,D]
        gate      = softmax(x @ gW[l] + gb[l])                    # [B,E]
        mixed     = einsum('be,bed->bd', gate, expert)            # [B,D]
        x         = x0 * mixed + bias[l] + x

Strategy (v3):
  - Data parallel: batch split 8 ways (2048 rows/core); params replicated.
  - Feature-on-partitions layout; host transposes x0 to [D, B_core].
  - Running-sum reformulation: x_l = x0*M_l + B_l, M_l = 1 + sum_{j<l} mixed_j,
    B_l = cumsum(bias) (const).  B_l folded into next-layer matmul biases
    (relu bias B@U, exp bias B@gW + gb); the FINAL B_2 is added on the host.
  - Packed gating: gW zero-padded to 32 columns; all 4 batch-tiles' logits
    land in ONE PSUM bank at partition offsets 0/32/64/96 (tile_position), so
    exp / sum / recip / cast / r4-broadcast / gate_n each run ONCE per layer
    instead of once per tile.  Sum and r4 use one-hot selector matmuls.
  - NO running-M tensor: the xb-chain identity  xb_{l+1} = xb_l + bf16(mix_l
    * x0)  (xb_1 = x0 + t_0, y = xb_2 + t_2) folds the running sum into the
    bf16 matmul-input tensor itself.  Every mixed element is touched exactly
    twice: one PSUM-side mul (t = mix*x0) and one bf16 add.
  - PSUM mixed banks are PAIR tiles [128,2,BT] (2 banks); drains alternate
    DVE-direct (t = mix*x0, PSUM port) and ACT-copy (mixsc = bf16(mix), bank
    freed, mul done later on DVE/Pool at bf16 rate) so banks free at ~2x the
    single-engine rate and the PE's V-phase never waits.
  - bf16 adds (xb update / y) are deferred and split DVE/GpSimd.
  - x0 for rep r+1 prefetched on the ACT DMA queue during rep r's layer 2
    (y-writes stay on the SP queue, so loads never queue behind stores).
  - Engine balance per rep: PE ~108us sim (510 matmuls, the dataflow minimum
    given 128-wide PE output chunks), DVE ~70, ACT ~48, Pool ~30, DMA ~23.
"""

import os
import sys

sys.path.insert(0, "/opt/trn_rl_repo")

import numpy as np

B, D, R, E, L = 16384, 1024, 64, 4, 3
N_CORES = 8
BC = B // N_CORES          # rows per core
BT = 512                   # batch-tile (free dim / PSUM bank width)
NBT = BC // BT             # b-tiles per core
ER = E * R                 # 256
DC = D // 128              # 8 d-chunks
EC = ER // 128             # 2 er-chunks
GP = 32                    # gW padded width (tile_position block)

# tiles whose deferred xb-adds ride GpSimd (early tiles have latency slack);
# the remaining tiles' adds use the faster DVE
ADD_POOL_TILES = int(os.environ.get("KADD_POOL_TILES", "2"))


def build_body(nc, tc, reps=1):
    import concourse.mybir as mybir

    f32 = mybir.dt.float32
    bf16 = mybir.dt.bfloat16
    AF = mybir.ActivationFunctionType

    x0T = nc.dram_tensor("x0T", [D, BC], bf16, kind="ExternalInput")
    U_all = nc.dram_tensor("U_all", [L, D, ER], bf16, kind="ExternalInput")
    V_all = nc.dram_tensor("V_all", [L, ER, D], bf16, kind="ExternalInput")
    gWp_all = nc.dram_tensor("gWp_all", [L, D, GP], bf16, kind="ExternalInput")
    sumsel_h = nc.dram_tensor("sumsel_h", [128, NBT], bf16, kind="ExternalInput")
    r4sel_h = nc.dram_tensor("r4sel_h", [NBT, 128], bf16, kind="ExternalInput")
    seltm_h = nc.dram_tensor("seltm_h", [128, NBT * EC, 128], bf16,
                             kind="ExternalInput")
    gbB_h = nc.dram_tensor("gbB_h", [128, L], f32, kind="ExternalInput")
    BU_h = nc.dram_tensor("BU_h", [128, L, EC], f32, kind="ExternalInput")
    yT = nc.dram_tensor("yT", [D, BC], bf16, kind="ExternalOutput")

    from contextlib import ExitStack

    ctx = ExitStack()
    const = ctx.enter_context(tc.tile_pool(name="const", bufs=1))
    x0p = ctx.enter_context(tc.tile_pool(name="x0p", bufs=NBT))
    xbp = ctx.enter_context(tc.tile_pool(name="xbp", bufs=NBT))
    explogp = ctx.enter_context(tc.tile_pool(name="explogp", bufs=2))
    gatenp = ctx.enter_context(tc.tile_pool(name="gatenp", bufs=2))
    smallp = ctx.enter_context(tc.tile_pool(name="smallp", bufs=2))
    hrp = ctx.enter_context(tc.tile_pool(name="hrp", bufs=6))
    hsp = ctx.enter_context(tc.tile_pool(name="hsp", bufs=4))
    tpp = ctx.enter_context(tc.tile_pool(name="tpp", bufs=8))
    mscp = ctx.enter_context(tc.tile_pool(name="mscp", bufs=6))
    tdp = ctx.enter_context(tc.tile_pool(name="tdp", bufs=4))
    ps_small = ctx.enter_context(tc.tile_pool(name="ps_small", bufs=1,
                                              space="PSUM"))
    # h-banks and gate-broadcast banks share one rotating pool
    ps_h = ctx.enter_context(tc.tile_pool(name="ps_h", bufs=3, space="PSUM"))
    # mixed single banks; drains alternate DVE/ACT per bank
    ps_mx = ctx.enter_context(tc.tile_pool(name="ps_mx", bufs=4, space="PSUM"))

    # ---- constants (loaded once, outside the reps loop) ----
    Ur = const.tile([128, L, DC, ER], bf16)
    Vr = const.tile([128, L, EC, D], bf16)
    gWpr = const.tile([128, L, DC, GP], bf16)
    sumsel = const.tile([128, NBT], bf16)
    r4sel = const.tile([NBT, 128], bf16)
    seltm = const.tile([128, NBT * EC, 128], bf16)
    gbBt = const.tile([128, L], f32)
    BUr = const.tile([128, L, EC], f32)

    nc.sync.dma_start(out=gWpr,
                      in_=gWp_all.rearrange("l (c p) e -> p l c e", p=128))
    nc.sync.dma_start(out=sumsel, in_=sumsel_h[:, :])
    nc.sync.dma_start(out=r4sel, in_=r4sel_h[:, :])
    nc.sync.dma_start(out=seltm, in_=seltm_h[:, :, :])
    nc.sync.dma_start(out=gbBt, in_=gbB_h[:, :])
    nc.sync.dma_start(out=BUr, in_=BU_h[:, :, :])
    for l in range(L):
        nc.sync.dma_start(
            out=Ur[:, l],
            in_=U_all[l].rearrange("(c p) er -> p c er", p=128),
        )
        nc.sync.dma_start(
            out=Vr[:, l],
            in_=V_all[l].rearrange("(k p) d -> p k d", p=128),
        )

    # persistent per-tile state
    x0_t = [x0p.tile([128, DC, BT], bf16, tag="x0", name=f"x0_{i}")
            for i in range(NBT)]
    xb_t = [xbp.tile([128, DC, BT], bf16, tag="xb", name=f"xb_{i}")
            for i in range(NBT)]

    state = {}

    def emit_x0_load(t):
        # Issued from the ACT queue so input loads never queue behind the
        # SP-queue y-writes.
        b0 = t * BT
        nc.scalar.dma_start(
            out=x0_t[t],
            in_=x0T[:, b0: b0 + BT].rearrange("(c p) b -> p c b", p=128),
        )

    def ph_logits(key, t):
        r, l = key
        xin = x0_t[t] if l == 0 else xb_t[t]
        if t == 0:
            lg = ps_small.tile([128, BT], f32, tag="sm", name="lg")
            state[key] = {"lg": lg, "hr": {}, "hs": {}}
        lg = state[key]["lg"]
        for c in range(DC):
            nc.tensor.matmul(
                lg[32 * t: 32 * t + 32, :],
                gWpr[:, l, c, :],
                xin[:, c, :],
                start=(c == 0),
                stop=(c == DC - 1),
                tile_position=(0, 32 * t),
            )

    def ph_exp(key):
        r, l = key
        st = state[key]
        explog = explogp.tile([128, BT], bf16, tag="explog")
        nc.scalar.activation(out=explog, in_=st["lg"], func=AF.Exp,
                             bias=gbBt[:, l: l + 1], scale=1.0)
        st["explog"] = explog

    def ph_sum(key):
        st = state[key]
        sa = ps_small.tile([NBT, BT], f32, tag="sm")
        nc.tensor.matmul(sa, sumsel[:, :], st["explog"], start=True, stop=True)
        recip = smallp.tile([NBT, BT], f32, tag="recip")
        nc.vector.reciprocal_approx_fast(out=recip, in_=sa)
        recipb = smallp.tile([NBT, BT], bf16, tag="recipb")
        nc.scalar.copy(out=recipb, in_=recip)
        st["recipb"] = recipb

    def ph_r4(key):
        st = state[key]
        r4 = ps_small.tile([128, BT], f32, tag="sm")
        nc.tensor.matmul(r4, r4sel[:, :], st["recipb"], start=True, stop=True)
        gaten = gatenp.tile([128, BT], bf16, tag="gaten")
        with nc.allow_low_precision(reason="gate in matmul dtype"):
            nc.vector.tensor_mul(out=gaten, in0=st["explog"], in1=r4)
        st["gaten"] = gaten

    def ph_U(key, t, m):
        r, l = key
        xin = x0_t[t] if l == 0 else xb_t[t]
        hm = ps_h.tile([128, BT], f32, tag="hp")
        for c in range(DC):
            nc.tensor.matmul(
                hm,
                Ur[:, l, c, m * 128: (m + 1) * 128],
                xin[:, c, :],
                start=(c == 0),
                stop=(c == DC - 1),
            )
        hr = hrp.tile([128, BT], bf16, tag="hr")
        nc.scalar.activation(out=hr, in_=hm, func=AF.Relu,
                             bias=BUr[:, l, m: m + 1], scale=1.0)
        state[key]["hr"][(t, m)] = hr

    def ph_gbc(key, t):
        st = state[key]
        hs = hsp.tile([128, EC, BT], bf16, tag="hs")
        for m in range(EC):
            gb = ps_h.tile([128, BT], f32, tag="hp", name="gb")
            nc.tensor.matmul(gb, seltm[:, EC * t + m, :], st["gaten"],
                             start=True, stop=True)
            with nc.allow_low_precision(reason="hs in matmul dtype"):
                nc.vector.tensor_mul(out=hs[:, m, :], in0=st["hr"].pop((t, m)),
                                     in1=gb)
        st["hs"][t] = hs

    def ph_V_mm(key, t):
        # V matmuls into single banks + the bank-freeing drain ONLY.
        # Drains alternate DVE-direct / ACT-copy per bank so banks free at
        # ~2x the single-engine elementwise rate; outputs land in HALVES of
        # pair tiles so the deferred bf16 post-work stays pair-fused.
        r, l = key
        st = state[key]
        hs = st["hs"].pop(t)
        tprs = st.setdefault("tprs", {})
        for p in range(DC // 2):           # 4 pairs = 8 single-bank groups
            d0 = 2 * p
            if p % 2 == 0:
                buf = tpp.tile([128, 2, BT], bf16, tag="tp", name="tp")
                kind = "t"
            else:
                buf = mscp.tile([128, 2, BT], bf16, tag="msc", name="msc")
                kind = "m"
            tprs[(t, p)] = (kind, buf)
            for dd in range(2):
                mx = ps_mx.tile([128, BT], f32, tag="mx", name="mx")
                for k in range(EC):
                    nc.tensor.matmul(
                        mx,
                        Vr[:, l, k, (d0 + dd) * 128: (d0 + dd + 1) * 128],
                        hs[:, k, :],
                        start=(k == 0),
                        stop=(k == EC - 1),
                    )
                if kind == "t":
                    # DVE-direct drain: t = mix * x0 straight from PSUM
                    with nc.allow_low_precision(reason="t in bf16"):
                        nc.vector.tensor_mul(
                            out=buf[:, dd, :], in0=mx,
                            in1=x0_t[t][:, d0 + dd, :])
                else:
                    # ACT-copy drain: bank freed by the copy; mul deferred
                    nc.scalar.copy(out=buf[:, dd, :], in_=mx)

    def ph_V_post(key, t):
        # deferred bf16 muls (for ACT-copied pairs) + xb-chain adds
        r, l = key
        b0 = t * BT
        st = state[key]
        last = l == L - 1
        for p in range(DC // 2):
            d0 = 2 * p
            x0pr = x0_t[t][:, d0: d0 + 2, :]
            kind, buf = st["tprs"].pop((t, p))
            if kind == "m":
                tpr = tpp.tile([128, 2, BT], bf16, tag="tp", name="tp")
                with nc.allow_low_precision(reason="t in bf16"):
                    nc.vector.tensor_mul(out=tpr, in0=buf, in1=x0pr)
            else:
                tpr = buf
            # adds: early tiles ride the idle GpSimd queue; late tiles use
            # the faster DVE so next-step logits aren't kept waiting
            eng = nc.gpsimd if t < ADD_POOL_TILES else nc.vector
            xbpr = xb_t[t][:, d0: d0 + 2, :]
            with nc.allow_low_precision(reason="xb chain in bf16"):
                if l == 0:
                    eng.tensor_add(out=xbpr, in0=x0pr, in1=tpr)
                elif l == 1:
                    eng.tensor_add(out=xbpr, in0=xbpr, in1=tpr)
                else:
                    td = tdp.tile([128, 2, BT], bf16, tag="td", name="td")
                    eng.tensor_add(out=td, in0=xbpr, in1=tpr)
                    nc.sync.dma_start(
                        out=yT[:, b0: b0 + BT].rearrange(
                            "(c p) b -> p c b", p=128)[:, d0: d0 + 2, :],
                        in_=td,
                    )
        if last:
            # prefetch next rep's x0 tile (same data every rep; ordering
            # after this rep's last x0 read keeps the race detector happy)
            emit_x0_load(t)

    def emit_step(key):
        # Emit order interleaves the cross-engine gating chain behind PE
        # GEMM work so each dependency has resolved when its consumer engine
        # reaches it.
        for t in range(NBT):
            ph_logits(key, t)
        ph_exp(key)
        ph_U(key, 0, 0)
        ph_U(key, 0, 1)
        ph_sum(key)
        ph_U(key, 1, 0)
        ph_U(key, 1, 1)
        ph_r4(key)
        ph_U(key, 2, 0)
        ph_gbc(key, 0)
        ph_U(key, 2, 1)
        ph_gbc(key, 1)
        ph_V_mm(key, 0)
        ph_U(key, 3, 0)
        ph_gbc(key, 2)
        ph_V_mm(key, 1)
        ph_V_post(key, 0)
        ph_U(key, 3, 1)
        ph_gbc(key, 3)
        ph_V_mm(key, 2)
        ph_V_post(key, 1)
        ph_V_mm(key, 3)
        ph_V_post(key, 2)
        ph_V_post(key, 3)
        state.pop(key)

    def pipeline(unroll):
        for r in range(unroll):
            for l in range(L):
                emit_step((r, l))

    def preamble():
        for t in range(NBT):
            emit_x0_load(t)

    if reps == 1:
        preamble()
        pipeline(1)
    elif os.environ.get("KFORCE_UNROLL"):
        preamble()
        pipeline(reps)
    else:
        unroll = next((u for u in (16, 8, 4, 2) if reps % u == 0), 1)
        preamble()
        with tc.For_i(0, reps // unroll, 1):
            pipeline(unroll)

    ctx.close()


def prep_inputs(x0, U, V, gW, gb, bias):
    """Host-side shard + transpose + param reshape. Returns list of per-core
    input dicts."""
    import ml_dtypes

    bf = ml_dtypes.bfloat16

    x0 = np.ascontiguousarray(np.asarray(x0, dtype=np.float32))
    U = np.asarray(U, dtype=np.float32)
    V = np.asarray(V, dtype=np.float32)
    gW = np.ascontiguousarray(np.asarray(gW, dtype=np.float32))
    gb = np.asarray(gb, dtype=np.float32)
    bias = np.asarray(bias, dtype=np.float32)

    U_all = np.ascontiguousarray(U.transpose(0, 2, 1, 3).reshape(L, D, ER))
    V_all = np.ascontiguousarray(V.reshape(L, ER, D))

    # B_l = sum_{j<=l} bias_j ; affine shifts folded into next-layer biases
    Bcum = np.cumsum(bias, axis=0)                     # [L, D], B_l
    Bprev = np.concatenate([np.zeros((1, D), np.float32), Bcum[:-1]], 0)

    # gW zero-padded to GP columns
    gWp = np.zeros((L, D, GP), np.float32)
    gWp[:, :, :E] = gW

    # exp bias rows 32t+e <- gb[l,e] + Bprev[l] @ gW[l][:,e]; rest 0
    gbB = np.stack([gb[l] + Bprev[l] @ gW[l] for l in range(L)], axis=1)  # [E,L]
    gbB_all = np.zeros((128, L), np.float32)
    for t in range(NBT):
        gbB_all[32 * t: 32 * t + E, :] = gbB

    # relu bias: B_{l-1} @ U_l  -> [L, ER] -> [128, L, EC]
    BU = np.stack([Bprev[l] @ U_all[l] for l in range(L)], axis=0)  # [L, ER]
    BU_t = np.ascontiguousarray(
        BU.reshape(L, EC, 128).transpose(2, 0, 1).astype(np.float32)
    )

    # selectors
    sumsel = np.zeros((128, NBT), np.float32)
    for t in range(NBT):
        for e in range(E):
            sumsel[32 * t + e, t] = 1.0
    r4sel = np.zeros((NBT, 128), np.float32)
    for p in range(128):
        r4sel[p // 32, p] = 1.0
    seltm = np.zeros((128, NBT * EC, 128), np.float32)
    for t in range(NBT):
        for m in range(EC):
            for p in range(128):
                seltm[32 * t + 2 * m + p // 64, EC * t + m, p] = 1.0

    shared = dict(
        U_all=U_all.astype(bf),
        V_all=V_all.astype(bf),
        gWp_all=gWp.astype(bf),
        sumsel_h=sumsel.astype(bf),
        r4sel_h=r4sel.astype(bf),
        seltm_h=seltm.astype(bf),
        gbB_h=gbB_all,
        BU_h=BU_t,
    )
    in_maps = []
    for c in range(N_CORES):
        shard = x0[c * BC: (c + 1) * BC, :]                           # [BC, D]
        x0Tc = np.ascontiguousarray(shard.T).astype(bf)               # [D, BC]
        m = dict(shared)
        m["x0T"] = x0Tc
        in_maps.append(m)
    return in_maps


_compiled = None


def _get_compiled():
    global _compiled
    if _compiled is None:
        import jax
        import numpy as _np
        from jax.sharding import Mesh, PartitionSpec, NamedSharding
        from jax.experimental.shard_map import shard_map
        import concourse.bacc as bacc
        import concourse.mybir as mybir
        from concourse import bass2jax
        from concourse.tile import TileContext

        nc = bacc.Bacc(
            "TRN2", target_bir_lowering=False, debug=False, num_devices=N_CORES
        )
        with TileContext(nc) as tc:
            build_body(nc, tc, reps=1)
        nc.compile()

        in_names, out_names, out_avals = [], [], []
        zero_shapes = []
        for alloc in nc.m.functions[0].allocations:
            if not isinstance(alloc, mybir.MemoryLocationSet):
                continue
            name = alloc.memorylocations[0].name
            if alloc.kind == "ExternalInput":
                in_names.append(name)
            elif alloc.kind == "ExternalOutput":
                out_names.append(name)
                shape = tuple(alloc.tensor_shape)
                dtype = mybir.dt.np(alloc.dtype)
                out_avals.append(jax.core.ShapedArray(shape, dtype))
                zero_shapes.append((shape, dtype))

        def _bass_body(*args):
            outs = bass2jax._bass_exec_p.bind(
                *args,
                out_avals=tuple(out_avals),
                in_names=tuple(in_names + out_names),
                out_names=tuple(out_names),
                lowering_input_output_aliases=(),
                sim_require_finite=True,
                sim_require_nnan=True,
                nc=nc,
            )
            return tuple(outs)

        devices = jax.devices()[:N_CORES]
        mesh = Mesh(_np.asarray(devices), ("core",))
        n_params, n_outs = len(in_names), len(out_names)
        fn = jax.jit(
            shard_map(
                _bass_body,
                mesh=mesh,
                in_specs=(PartitionSpec("core"),) * (n_params + n_outs),
                out_specs=(PartitionSpec("core"),) * n_outs,
                check_rep=False,
            ),
            keep_unused=True,
        )
        sharding = NamedSharding(mesh, PartitionSpec("core"))
        _compiled = (fn, in_names, out_names, out_avals, zero_shapes, sharding)
    return _compiled


def run_device(in_maps):
    """Run the compiled NEFF on 8 cores; returns list of per-core out dicts."""
    import jax

    fn, in_names, out_names, out_avals, zero_shapes, sharding = _get_compiled()
    concat = []
    for n in in_names:
        if n == "partition_id":
            concat.append(
                np.arange(N_CORES, dtype=np.uint32).reshape(N_CORES, 1)
            )
        else:
            concat.append(
                np.concatenate([np.asarray(m[n]) for m in in_maps], axis=0)
            )
    zeros = [np.zeros((N_CORES * s[0], *s[1:]), d) for (s, d) in zero_shapes]
    dev_args = [jax.device_put(a, sharding) for a in concat + zeros]
    outs = fn(*dev_args)
    jax.block_until_ready(outs)
    res = []
    for c in range(N_CORES):
        d = {}
        for i, name in enumerate(out_names):
            shape = out_avals[i].shape
            d[name] = np.asarray(outs[i]).reshape(N_CORES, *shape)[c]
        res.append(d)
    return res


def kernel(x0, U, V, gW, gb, bias):
    in_maps = prep_inputs(x0, U, V, gW, gb, bias)
    res = run_device(in_maps)
    bias = np.asarray(bias, dtype=np.float32)
    B2 = np.cumsum(bias, axis=0)[L - 1]                 # final-layer bias [D]
    out = np.empty((B, D), np.float32)
    for c in range(N_CORES):
        out[c * BC: (c + 1) * BC, :] = (
            res[c]["yT"].astype(np.float32).T + B2[None, :]
        )
    return out


# revision 13
# speedup vs baseline: 1.2553x; 1.0387x over previous
"""CrossNetV2 soft-MoE kernel for 8 axon-tunneled TRN2 NeuronCores (v2).

Problem (hardcoded shapes): B=16384, D=1024, R=64, E=4, L=3.
    for l in range(L):
        h         = relu(einsum('bd,edr->ber', x, U[l]))          # [B,E,R]
        expert    = einsum('ber,erd->bed', h, V[l])               # [B,E,D]
        gate      = softmax(x @ gW[l] + gb[l])                    # [B,E]
        mixed     = einsum('be,bed->bd', gate, expert)            # [B,D]
        x         = x0 * mixed + bias[l] + x

Strategy (v3):
  - Data parallel: batch split 8 ways (2048 rows/core); params replicated.
  - Feature-on-partitions layout; host transposes x0 to [D, B_core].
  - Running-sum reformulation: x_l = x0*M_l + B_l, M_l = 1 + sum_{j<l} mixed_j,
    B_l = cumsum(bias) (const).  B_l folded into next-layer matmul biases
    (relu bias B@U, exp bias B@gW + gb); the FINAL B_2 is added on the host.
  - Packed gating: gW zero-padded to 32 columns; all 4 batch-tiles' logits
    land in ONE PSUM bank at partition offsets 0/32/64/96 (tile_position), so
    exp / sum / recip / cast / r4-broadcast / gate_n each run ONCE per layer
    instead of once per tile.  Sum and r4 use one-hot selector matmuls.
  - NO running-M tensor: the xb-chain identity  xb_{l+1} = xb_l + bf16(mix_l
    * x0)  (xb_1 = x0 + t_0, y = xb_2 + t_2) folds the running sum into the
    bf16 matmul-input tensor itself.  Every mixed element is touched exactly
    twice: one PSUM-side mul (t = mix*x0) and one bf16 add.
  - PSUM mixed banks are PAIR tiles [128,2,BT] (2 banks); drains alternate
    DVE-direct (t = mix*x0, PSUM port) and ACT-copy (mixsc = bf16(mix), bank
    freed, mul done later on DVE/Pool at bf16 rate) so banks free at ~2x the
    single-engine rate and the PE's V-phase never waits.
  - bf16 adds (xb update / y) are deferred and split DVE/GpSimd.
  - x0 for rep r+1 prefetched on the ACT DMA queue during rep r's layer 2
    (y-writes stay on the SP queue, so loads never queue behind stores).
  - Engine balance per rep: PE ~108us sim (510 matmuls, the dataflow minimum
    given 128-wide PE output chunks), DVE ~70, ACT ~48, Pool ~30, DMA ~23.
"""

import os
import sys

sys.path.insert(0, "/opt/trn_rl_repo")

import numpy as np

B, D, R, E, L = 16384, 1024, 64, 4, 3
N_CORES = 8
BC = B // N_CORES          # rows per core
BT = 512                   # batch-tile (free dim / PSUM bank width)
NBT = BC // BT             # b-tiles per core
ER = E * R                 # 256
DC = D // 128              # 8 d-chunks
EC = ER // 128             # 2 er-chunks
GP = 32                    # gW padded width (tile_position block)

# tiles whose deferred xb-adds ride GpSimd (early tiles have latency slack);
# the remaining tiles' adds use the faster DVE
ADD_POOL_TILES = int(os.environ.get("KADD_POOL_TILES", "0"))
# tiles whose deferred bf16 muls (ACT-copied drains) ride GpSimd
MUL_POOL_TILES = int(os.environ.get("KMUL_POOL_TILES", "3"))


def build_body(nc, tc, reps=1):
    import concourse.mybir as mybir

    f32 = mybir.dt.float32
    bf16 = mybir.dt.bfloat16
    AF = mybir.ActivationFunctionType

    x0T = nc.dram_tensor("x0T", [D, BC], bf16, kind="ExternalInput")
    U_all = nc.dram_tensor("U_all", [L, D, ER], bf16, kind="ExternalInput")
    V_all = nc.dram_tensor("V_all", [L, ER, D], bf16, kind="ExternalInput")
    gWp_all = nc.dram_tensor("gWp_all", [L, D, GP], bf16, kind="ExternalInput")
    sumsel_h = nc.dram_tensor("sumsel_h", [128, NBT], bf16, kind="ExternalInput")
    r4sel_h = nc.dram_tensor("r4sel_h", [NBT, 128], bf16, kind="ExternalInput")
    seltm_h = nc.dram_tensor("seltm_h", [128, NBT * EC, 128], bf16,
                             kind="ExternalInput")
    gbB_h = nc.dram_tensor("gbB_h", [128, L], f32, kind="ExternalInput")
    BU_h = nc.dram_tensor("BU_h", [128, L, EC], f32, kind="ExternalInput")
    yT = nc.dram_tensor("yT", [D, BC], bf16, kind="ExternalOutput")

    from contextlib import ExitStack

    ctx = ExitStack()
    const = ctx.enter_context(tc.tile_pool(name="const", bufs=1))
    x0p = ctx.enter_context(tc.tile_pool(name="x0p", bufs=NBT))
    xbp = ctx.enter_context(tc.tile_pool(name="xbp", bufs=NBT))
    explogp = ctx.enter_context(tc.tile_pool(name="explogp", bufs=2))
    gatenp = ctx.enter_context(tc.tile_pool(name="gatenp", bufs=2))
    smallp = ctx.enter_context(tc.tile_pool(name="smallp", bufs=2))
    hrp = ctx.enter_context(tc.tile_pool(name="hrp", bufs=6))
    hsp = ctx.enter_context(tc.tile_pool(name="hsp", bufs=4))
    tpp = ctx.enter_context(tc.tile_pool(name="tpp", bufs=8))
    mscp = ctx.enter_context(tc.tile_pool(name="mscp", bufs=6))
    tdp = ctx.enter_context(tc.tile_pool(name="tdp", bufs=4))
    ps_small = ctx.enter_context(tc.tile_pool(name="ps_small", bufs=1,
                                              space="PSUM"))
    # h-banks and gate-broadcast banks share one rotating pool
    ps_h = ctx.enter_context(tc.tile_pool(name="ps_h", bufs=3, space="PSUM"))
    # mixed single banks; drains alternate DVE/ACT per bank
    ps_mx = ctx.enter_context(tc.tile_pool(name="ps_mx", bufs=4, space="PSUM"))

    # ---- constants (loaded once, outside the reps loop) ----
    Ur = const.tile([128, L, DC, ER], bf16)
    Vr = const.tile([128, L, EC, D], bf16)
    gWpr = const.tile([128, L, DC, GP], bf16)
    sumsel = const.tile([128, NBT], bf16)
    r4sel = const.tile([NBT, 128], bf16)
    seltm = const.tile([128, NBT * EC, 128], bf16)
    gbBt = const.tile([128, L], f32)
    BUr = const.tile([128, L, EC], f32)

    nc.sync.dma_start(out=gWpr,
                      in_=gWp_all.rearrange("l (c p) e -> p l c e", p=128))
    nc.sync.dma_start(out=sumsel, in_=sumsel_h[:, :])
    nc.sync.dma_start(out=r4sel, in_=r4sel_h[:, :])
    nc.sync.dma_start(out=seltm, in_=seltm_h[:, :, :])
    nc.sync.dma_start(out=gbBt, in_=gbB_h[:, :])
    nc.sync.dma_start(out=BUr, in_=BU_h[:, :, :])
    for l in range(L):
        nc.sync.dma_start(
            out=Ur[:, l],
            in_=U_all[l].rearrange("(c p) er -> p c er", p=128),
        )
        nc.sync.dma_start(
            out=Vr[:, l],
            in_=V_all[l].rearrange("(k p) d -> p k d", p=128),
        )

    # persistent per-tile state
    x0_t = [x0p.tile([128, DC, BT], bf16, tag="x0", name=f"x0_{i}")
            for i in range(NBT)]
    xb_t = [xbp.tile([128, DC, BT], bf16, tag="xb", name=f"xb_{i}")
            for i in range(NBT)]

    state = {}

    def emit_x0_load(t):
        # Issued from the ACT queue so input loads never queue behind the
        # SP-queue y-writes.
        b0 = t * BT
        nc.scalar.dma_start(
            out=x0_t[t],
            in_=x0T[:, b0: b0 + BT].rearrange("(c p) b -> p c b", p=128),
        )

    def ph_logits(key, t):
        r, l = key
        xin = x0_t[t] if l == 0 else xb_t[t]
        if t == 0:
            lg = ps_small.tile([128, BT], f32, tag="sm", name="lg")
            state[key] = {"lg": lg, "hr": {}, "hs": {}}
        lg = state[key]["lg"]
        for c in range(DC):
            nc.tensor.matmul(
                lg[32 * t: 32 * t + 32, :],
                gWpr[:, l, c, :],
                xin[:, c, :],
                start=(c == 0),
                stop=(c == DC - 1),
                tile_position=(0, 32 * t),
            )

    def ph_exp(key):
        r, l = key
        st = state[key]
        explog = explogp.tile([128, BT], bf16, tag="explog")
        nc.scalar.activation(out=explog, in_=st["lg"], func=AF.Exp,
                             bias=gbBt[:, l: l + 1], scale=1.0)
        st["explog"] = explog

    def ph_sum(key):
        st = state[key]
        sa = ps_small.tile([NBT, BT], f32, tag="sm")
        nc.tensor.matmul(sa, sumsel[:, :], st["explog"], start=True, stop=True)
        recip = smallp.tile([NBT, BT], f32, tag="recip")
        nc.vector.reciprocal_approx_fast(out=recip, in_=sa)
        recipb = smallp.tile([NBT, BT], bf16, tag="recipb")
        nc.scalar.copy(out=recipb, in_=recip)
        st["recipb"] = recipb

    def ph_r4(key):
        st = state[key]
        r4 = ps_small.tile([128, BT], f32, tag="sm")
        nc.tensor.matmul(r4, r4sel[:, :], st["recipb"], start=True, stop=True)
        gaten = gatenp.tile([128, BT], bf16, tag="gaten")
        with nc.allow_low_precision(reason="gate in matmul dtype"):
            nc.vector.tensor_mul(out=gaten, in0=st["explog"], in1=r4)
        st["gaten"] = gaten

    def ph_U(key, t, m):
        r, l = key
        xin = x0_t[t] if l == 0 else xb_t[t]
        hm = ps_h.tile([128, BT], f32, tag="hp")
        for c in range(DC):
            nc.tensor.matmul(
                hm,
                Ur[:, l, c, m * 128: (m + 1) * 128],
                xin[:, c, :],
                start=(c == 0),
                stop=(c == DC - 1),
            )
        hr = hrp.tile([128, BT], bf16, tag="hr")
        nc.scalar.activation(out=hr, in_=hm, func=AF.Relu,
                             bias=BUr[:, l, m: m + 1], scale=1.0)
        state[key]["hr"][(t, m)] = hr

    def ph_gbc(key, t):
        st = state[key]
        hs = hsp.tile([128, EC, BT], bf16, tag="hs")
        for m in range(EC):
            gb = ps_h.tile([128, BT], f32, tag="hp", name="gb")
            nc.tensor.matmul(gb, seltm[:, EC * t + m, :], st["gaten"],
                             start=True, stop=True)
            with nc.allow_low_precision(reason="hs in matmul dtype"):
                nc.vector.tensor_mul(out=hs[:, m, :], in0=st["hr"].pop((t, m)),
                                     in1=gb)
        st["hs"][t] = hs

    def ph_V_mm(key, t):
        # V matmuls into single banks + the bank-freeing drain ONLY.
        # Drains alternate DVE-direct / ACT-copy per bank so banks free at
        # ~2x the single-engine elementwise rate; outputs land in HALVES of
        # pair tiles so the deferred bf16 post-work stays pair-fused.
        r, l = key
        st = state[key]
        hs = st["hs"].pop(t)
        tprs = st.setdefault("tprs", {})
        for p in range(DC // 2):           # 4 pairs = 8 single-bank groups
            d0 = 2 * p
            if p % 2 == 0:
                buf = tpp.tile([128, 2, BT], bf16, tag="tp", name="tp")
                kind = "t"
            else:
                buf = mscp.tile([128, 2, BT], bf16, tag="msc", name="msc")
                kind = "m"
            tprs[(t, p)] = (kind, buf)
            for dd in range(2):
                mx = ps_mx.tile([128, BT], f32, tag="mx", name="mx")
                for k in range(EC):
                    nc.tensor.matmul(
                        mx,
                        Vr[:, l, k, (d0 + dd) * 128: (d0 + dd + 1) * 128],
                        hs[:, k, :],
                        start=(k == 0),
                        stop=(k == EC - 1),
                    )
                if kind == "t":
                    # DVE-direct drain: t = mix * x0 straight from PSUM
                    with nc.allow_low_precision(reason="t in bf16"):
                        nc.vector.tensor_mul(
                            out=buf[:, dd, :], in0=mx,
                            in1=x0_t[t][:, d0 + dd, :])
                else:
                    # ACT-copy drain: bank freed by the copy; mul deferred
                    nc.scalar.copy(out=buf[:, dd, :], in_=mx)

    def ph_V_post(key, t):
        # deferred bf16 muls (for ACT-copied pairs) + xb-chain adds
        r, l = key
        b0 = t * BT
        st = state[key]
        last = l == L - 1
        for p in range(DC // 2):
            d0 = 2 * p
            x0pr = x0_t[t][:, d0: d0 + 2, :]
            kind, buf = st["tprs"].pop((t, p))
            if kind == "m":
                tpr = tpp.tile([128, 2, BT], bf16, tag="tp", name="tp")
                meng = nc.gpsimd if t < MUL_POOL_TILES else nc.vector
                with nc.allow_low_precision(reason="t in bf16"):
                    meng.tensor_mul(out=tpr, in0=buf, in1=x0pr)
            else:
                tpr = buf
            # adds: early tiles ride the idle GpSimd queue; late tiles use
            # the faster DVE so next-step logits aren't kept waiting
            eng = nc.gpsimd if t < ADD_POOL_TILES else nc.vector
            xbpr = xb_t[t][:, d0: d0 + 2, :]
            with nc.allow_low_precision(reason="xb chain in bf16"):
                if l == 0:
                    eng.tensor_add(out=xbpr, in0=x0pr, in1=tpr)
                elif l == 1:
                    eng.tensor_add(out=xbpr, in0=xbpr, in1=tpr)
                else:
                    td = tdp.tile([128, 2, BT], bf16, tag="td", name="td")
                    eng.tensor_add(out=td, in0=xbpr, in1=tpr)
                    nc.sync.dma_start(
                        out=yT[:, b0: b0 + BT].rearrange(
                            "(c p) b -> p c b", p=128)[:, d0: d0 + 2, :],
                        in_=td,
                    )
        if last:
            # prefetch next rep's x0 tile (same data every rep; ordering
            # after this rep's last x0 read keeps the race detector happy)
            emit_x0_load(t)

    def emit_step(key):
        # Emit order interleaves the cross-engine gating chain behind PE
        # GEMM work so each dependency has resolved when its consumer engine
        # reaches it.
        for t in range(NBT):
            ph_logits(key, t)
        ph_U(key, 0, 0)
        ph_exp(key)
        ph_U(key, 0, 1)
        ph_sum(key)
        ph_U(key, 1, 0)
        ph_U(key, 1, 1)
        ph_r4(key)
        ph_U(key, 2, 0)
        ph_gbc(key, 0)
        ph_U(key, 2, 1)
        ph_gbc(key, 1)
        ph_V_mm(key, 0)
        ph_U(key, 3, 0)
        ph_gbc(key, 2)
        ph_V_mm(key, 1)
        ph_V_post(key, 0)
        ph_U(key, 3, 1)
        ph_gbc(key, 3)
        ph_V_mm(key, 2)
        ph_V_post(key, 1)
        ph_V_mm(key, 3)
        ph_V_post(key, 2)
        ph_V_post(key, 3)
        state.pop(key)

    def pipeline(unroll):
        for r in range(unroll):
            for l in range(L):
                emit_step((r, l))

    def preamble():
        for t in range(NBT):
            emit_x0_load(t)

    if reps == 1:
        preamble()
        pipeline(1)
    elif os.environ.get("KFORCE_UNROLL"):
        preamble()
        pipeline(reps)
    else:
        unroll = next((u for u in (16, 8, 4, 2) if reps % u == 0), 1)
        preamble()
        with tc.For_i(0, reps // unroll, 1):
            pipeline(unroll)

    ctx.close()


def prep_inputs(x0, U, V, gW, gb, bias):
    """Host-side shard + transpose + param reshape. Returns list of per-core
    input dicts."""
    import ml_dtypes

    bf = ml_dtypes.bfloat16

    x0 = np.ascontiguousarray(np.asarray(x0, dtype=np.float32))
    U = np.asarray(U, dtype=np.float32)
    V = np.asarray(V, dtype=np.float32)
    gW = np.ascontiguousarray(np.asarray(gW, dtype=np.float32))
    gb = np.asarray(gb, dtype=np.float32)
    bias = np.asarray(bias, dtype=np.float32)

    U_all = np.ascontiguousarray(U.transpose(0, 2, 1, 3).reshape(L, D, ER))
    V_all = np.ascontiguousarray(V.reshape(L, ER, D))

    # B_l = sum_{j<=l} bias_j ; affine shifts folded into next-layer biases
    Bcum = np.cumsum(bias, axis=0)                     # [L, D], B_l
    Bprev = np.concatenate([np.zeros((1, D), np.float32), Bcum[:-1]], 0)

    # gW zero-padded to GP columns
    gWp = np.zeros((L, D, GP), np.float32)
    gWp[:, :, :E] = gW

    # exp bias rows 32t+e <- gb[l,e] + Bprev[l] @ gW[l][:,e]; rest 0
    gbB = np.stack([gb[l] + Bprev[l] @ gW[l] for l in range(L)], axis=1)  # [E,L]
    gbB_all = np.zeros((128, L), np.float32)
    for t in range(NBT):
        gbB_all[32 * t: 32 * t + E, :] = gbB

    # relu bias: B_{l-1} @ U_l  -> [L, ER] -> [128, L, EC]
    BU = np.stack([Bprev[l] @ U_all[l] for l in range(L)], axis=0)  # [L, ER]
    BU_t = np.ascontiguousarray(
        BU.reshape(L, EC, 128).transpose(2, 0, 1).astype(np.float32)
    )

    # selectors
    sumsel = np.zeros((128, NBT), np.float32)
    for t in range(NBT):
        for e in range(E):
            sumsel[32 * t + e, t] = 1.0
    r4sel = np.zeros((NBT, 128), np.float32)
    for p in range(128):
        r4sel[p // 32, p] = 1.0
    seltm = np.zeros((128, NBT * EC, 128), np.float32)
    for t in range(NBT):
        for m in range(EC):
            for p in range(128):
                seltm[32 * t + 2 * m + p // 64, EC * t + m, p] = 1.0

    shared = dict(
        U_all=U_all.astype(bf),
        V_all=V_all.astype(bf),
        gWp_all=gWp.astype(bf),
        sumsel_h=sumsel.astype(bf),
        r4sel_h=r4sel.astype(bf),
        seltm_h=seltm.astype(bf),
        gbB_h=gbB_all,
        BU_h=BU_t,
    )
    in_maps = []
    for c in range(N_CORES):
        shard = x0[c * BC: (c + 1) * BC, :]                           # [BC, D]
        x0Tc = np.ascontiguousarray(shard.T).astype(bf)               # [D, BC]
        m = dict(shared)
        m["x0T"] = x0Tc
        in_maps.append(m)
    return in_maps


_compiled = None


def _get_compiled():
    global _compiled
    if _compiled is None:
        import jax
        import numpy as _np
        from jax.sharding import Mesh, PartitionSpec, NamedSharding
        from jax.experimental.shard_map import shard_map
        import concourse.bacc as bacc
        import concourse.mybir as mybir
        from concourse import bass2jax
        from concourse.tile import TileContext

        nc = bacc.Bacc(
            "TRN2", target_bir_lowering=False, debug=False, num_devices=N_CORES
        )
        with TileContext(nc) as tc:
            build_body(nc, tc, reps=1)
        nc.compile()

        in_names, out_names, out_avals = [], [], []
        zero_shapes = []
        for alloc in nc.m.functions[0].allocations:
            if not isinstance(alloc, mybir.MemoryLocationSet):
                continue
            name = alloc.memorylocations[0].name
            if alloc.kind == "ExternalInput":
                in_names.append(name)
            elif alloc.kind == "ExternalOutput":
                out_names.append(name)
                shape = tuple(alloc.tensor_shape)
                dtype = mybir.dt.np(alloc.dtype)
                out_avals.append(jax.core.ShapedArray(shape, dtype))
                zero_shapes.append((shape, dtype))

        def _bass_body(*args):
            outs = bass2jax._bass_exec_p.bind(
                *args,
                out_avals=tuple(out_avals),
                in_names=tuple(in_names + out_names),
                out_names=tuple(out_names),
                lowering_input_output_aliases=(),
                sim_require_finite=True,
                sim_require_nnan=True,
                nc=nc,
            )
            return tuple(outs)

        devices = jax.devices()[:N_CORES]
        mesh = Mesh(_np.asarray(devices), ("core",))
        n_params, n_outs = len(in_names), len(out_names)
        fn = jax.jit(
            shard_map(
                _bass_body,
                mesh=mesh,
                in_specs=(PartitionSpec("core"),) * (n_params + n_outs),
                out_specs=(PartitionSpec("core"),) * n_outs,
                check_rep=False,
            ),
            keep_unused=True,
        )
        sharding = NamedSharding(mesh, PartitionSpec("core"))
        _compiled = (fn, in_names, out_names, out_avals, zero_shapes, sharding)
    return _compiled


def run_device(in_maps):
    """Run the compiled NEFF on 8 cores; returns list of per-core out dicts."""
    import jax

    fn, in_names, out_names, out_avals, zero_shapes, sharding = _get_compiled()
    concat = []
    for n in in_names:
        if n == "partition_id":
            concat.append(
                np.arange(N_CORES, dtype=np.uint32).reshape(N_CORES, 1)
            )
        else:
            concat.append(
                np.concatenate([np.asarray(m[n]) for m in in_maps], axis=0)
            )
    zeros = [np.zeros((N_CORES * s[0], *s[1:]), d) for (s, d) in zero_shapes]
    dev_args = [jax.device_put(a, sharding) for a in concat + zeros]
    outs = fn(*dev_args)
    jax.block_until_ready(outs)
    res = []
    for c in range(N_CORES):
        d = {}
        for i, name in enumerate(out_names):
            shape = out_avals[i].shape
            d[name] = np.asarray(outs[i]).reshape(N_CORES, *shape)[c]
        res.append(d)
    return res


def kernel(x0, U, V, gW, gb, bias):
    in_maps = prep_inputs(x0, U, V, gW, gb, bias)
    res = run_device(in_maps)
    bias = np.asarray(bias, dtype=np.float32)
    B2 = np.cumsum(bias, axis=0)[L - 1]                 # final-layer bias [D]
    out = np.empty((B, D), np.float32)
    for c in range(N_CORES):
        out[c * BC: (c + 1) * BC, :] = (
            res[c]["yT"].astype(np.float32).T + B2[None, :]
        )
    return out
